# revision 1
# baseline (speedup 1.0000x reference)
"""Trainium2 Bass kernel for EnhancedMultiHeadSelfAttention (dense transformer block).

Sharding: sequence-parallel over 8 cores. Each core owns L/8 = 256 query rows.
LN1 + K/V projection for all 2048 tokens are replicated on every core (cheaper
than on-chip AllReduce at this size); scores/softmax/attn@V/out-proj/LN2/FFN are
computed only for the core's own 256 rows. No collectives.

Layout: activations are kept feature-major ("transposed", [feature, token]) so
every linear layer is matmul(out=[cols, tok], lhsT=W[k,cols], rhs=actT[k,tok])
with natural weight layout and no on-device transposes. All matmuls run as
float32r (full fp32 data, bf16-rate PE throughput for free dim >= 256).

Math notes:
 - clip(scores,-10,10) never binds: |cos|*0.125 + bias in [-0.125, 0.225].
 - softmax needs no max-subtraction for the same reason.
 - the query-side half of the lcc bias is a per-query constant factor in
   exp-space and cancels in softmax normalization; only the key-side half is
   applied (as per-partition ACT bias in the exp).
 - softmax denominators come from an appended ones-column in V.
 - LN gains/biases are folded into the following matmul's weights on the host.
"""

import numpy as np

import concourse.bass as bass
import concourse.tile as tile
from concourse import bacc, mybir
from concourse.bass_utils import run_bass_kernel_spmd

F32 = mybir.dt.float32
F32R = mybir.dt.float32r

L = 2048          # sequence length
D = 1024          # model dim
H = 16            # heads
DH = 64           # head dim
FF = 4096         # ffn hidden
P = 128           # partitions
NCORES = 8
LQ = L // NCORES  # 256 own query rows per core
DC = D // P       # 8 d-model chunks
FC = FF // P      # 32 ffn chunks
KC = L // P       # 16 key chunks
NBLK = 4          # token blocks of 512 for the replicated phase
BLK = L // NBLK   # 512

# CoreSim doesn't implement Gelu; test_sim swaps this to Identity and checks
# against a gelu-less reference. Hardware always uses the real (erf) Gelu.
GELU_FUNC = mybir.ActivationFunctionType.Gelu

LN_EPS = 1e-5
NORM_EPS = 1e-12
SCALING = DH ** -0.5
LCC = 0.1


def _mm(nc, out, lhsT, rhs, start, stop):
    assert lhsT.dtype == F32R and rhs.dtype == F32R, (lhsT.dtype, rhs.dtype)
    nc.tensor.matmul(out, lhsT, rhs, start=start, stop=stop)


def emit(tc):
    nc = tc.nc

    xt = nc.dram_tensor("xt", [D, L], F32R, kind="ExternalInput").ap()
    xot = nc.dram_tensor("xot", [D, LQ], F32R, kind="ExternalInput").ap()
    wq = nc.dram_tensor("wq", [D, D], F32R, kind="ExternalInput").ap()
    wk = nc.dram_tensor("wk", [D, D], F32R, kind="ExternalInput").ap()
    wv = nc.dram_tensor("wv", [D, D], F32R, kind="ExternalInput").ap()
    wo = nc.dram_tensor("wo", [D, D], F32R, kind="ExternalInput").ap()
    wf1 = nc.dram_tensor("wf1", [D, FF], F32R, kind="ExternalInput").ap()
    wf2 = nc.dram_tensor("wf2", [FF, D], F32R, kind="ExternalInput").ap()
    bq = nc.dram_tensor("bq", [P, DC], F32, kind="ExternalInput").ap()
    bk = nc.dram_tensor("bk", [P, DC], F32, kind="ExternalInput").ap()
    bv = nc.dram_tensor("bv", [D], F32, kind="ExternalInput").ap()
    bo = nc.dram_tensor("bo", [P, DC], F32, kind="ExternalInput").ap()
    bf1 = nc.dram_tensor("bf1", [P, FC], F32, kind="ExternalInput").ap()
    bf2 = nc.dram_tensor("bf2", [P, DC], F32, kind="ExternalInput").ap()
    lcck = nc.dram_tensor("lcck", [P, KC], F32, kind="ExternalInput").ap()
    selr = nc.dram_tensor("selr", [P, P], F32R, kind="ExternalInput").ap()
    selb = nc.dram_tensor("selb", [H, DC * P], F32R, kind="ExternalInput").ap()
    onesc = nc.dram_tensor("onesc", [P, 3], F32R, kind="ExternalInput").ap()
    ones1r = nc.dram_tensor("ones1r", [1, P], F32R, kind="ExternalInput").ap()
    vones = nc.dram_tensor("vones", [P, KC], F32R, kind="ExternalInput").ap()
    out_t = nc.dram_tensor("out_t", [D, LQ], F32, kind="ExternalOutput").ap()

    xt3 = xt.rearrange("(c p) t -> p c t", p=P)        # [128, 8, 2048]
    xot3 = xot.rearrange("(c p) t -> p c t", p=P)      # [128, 8, 256]
    wq3 = wq.rearrange("(c p) n -> p c n", p=P)        # [128, 8, 1024]
    wk3 = wk.rearrange("(c p) n -> p c n", p=P)
    wv3 = wv.rearrange("(c p) n -> p c n", p=P)
    wo3 = wo.rearrange("(c p) n -> p c n", p=P)
    wf13 = wf1.rearrange("(c p) n -> p c n", p=P)      # [128, 8, 4096]
    wf23 = wf2.rearrange("(c p) n -> p c n", p=P)      # [128, 32, 1024]
    out3 = out_t.rearrange("(c p) t -> p c t", p=P)    # [128, 8, 256]

    # ---- persistent small constants -------------------------------------
    singles = tc.alloc_tile_pool(name="singles", bufs=1)
    ones_1x128 = singles.tile([1, P], F32R)  # K=1 broadcast lhsT
    nc.sync.dma_start(ones_1x128, ones1r)
    onesc_sb = singles.tile([P, 3], F32R)
    nc.sync.dma_start(onesc_sb, onesc)
    ones_col = onesc_sb[:, 0:1]              # K=128 -> M=1 reduction lhsT
    # head-norm selectors (host-precomputed):
    # selr_sb[:, m, h] = 1 if head h belongs to chunk m at this partition;
    # selb_sb[h, m*128+p] = transpose, for broadcasting norms back to chunks
    selr_sb = singles.tile([P, DC, H], F32R)
    nc.sync.dma_start(selr_sb, selr.rearrange("p (m h) -> p m h", h=H))
    selb_sb = singles.tile([H, DC, P], F32R)
    nc.sync.dma_start(selb_sb, selb.rearrange("h (m p) -> h m p", p=P))
    vones_sb = singles.tile([P, KC], F32R)
    nc.sync.dma_start(vones_sb, vones)
    bq_sb = singles.tile([P, DC], F32)
    nc.sync.dma_start(bq_sb, bq)
    bk_sb = singles.tile([P, DC], F32)
    nc.sync.dma_start(bk_sb, bk)
    bo_sb = singles.tile([P, DC], F32)
    nc.sync.dma_start(bo_sb, bo)
    bf1_sb = singles.tile([P, FC], F32)
    nc.sync.dma_start(bf1_sb, bf1)
    bf2_sb = singles.tile([P, DC], F32)
    nc.sync.dma_start(bf2_sb, bf2)
    lcc_sb = singles.tile([P, KC], F32)
    nc.sync.dma_start(lcc_sb, lcck)
    bv_sb = singles.tile([P, D], F32)  # b_v broadcast to all partitions
    nc.sync.dma_start(bv_sb, bass.AP(tensor=bv.tensor, offset=0, ap=[[0, P], [1, D]]))
    eps_sb = singles.tile([1, 1], F32)
    nc.vector.memset(eps_sb, LN_EPS)

    def layer_norm_t(ctx_pool, ps_stat, ps_coef, src_tiles, dst, ncols, sq_pool,
                     src3=None, dst3=None, add_eng=None):
        """LayerNorm along feature dim for feature-major tiles.

        src_tiles: list of DC tiles/APs [128, ncols] (feature chunks)
        dst: [128, DC, ncols] output tile
        """
        sums = ps_stat.tile([1, ncols], F32, tag="stat")
        sumsq = ps_stat.tile([1, ncols], F32, tag="stat")
        for c in range(DC):
            xc = src_tiles[c]
            xsq = sq_pool.tile([P, ncols], F32R, tag="xsq")
            nc.scalar.square(xsq, xc)
            _mm(nc, sums, ones_col, xc, c == 0, c == DC - 1)
            _mm(nc, sumsq, ones_col, xsq, c == 0, c == DC - 1)
        # coeffs on one partition: rstd, shift = -mu*rstd
        mu = ctx_pool.tile([1, ncols], F32, tag="mu")
        nc.vector.tensor_scalar_mul(mu, sums, 1.0 / D)
        ex2 = ctx_pool.tile([1, ncols], F32, tag="ex2")
        nc.vector.tensor_scalar_mul(ex2, sumsq, 1.0 / D)
        var = ctx_pool.tile([1, ncols], F32, tag="var")
        nc.vector.tensor_mul(var, mu, mu)
        nc.vector.tensor_sub(var, ex2, var)
        sd = ctx_pool.tile([1, ncols], F32, tag="sd")
        nc.scalar.activation(sd, var, func=mybir.ActivationFunctionType.Sqrt,
                             bias=eps_sb, scale=1.0)
        rstd = ctx_pool.tile([1, ncols], F32R, tag="rstd")
        with nc.allow_low_precision(reason="f32r matmul operand"):
            nc.vector.reciprocal(rstd, sd)
        shift = ctx_pool.tile([1, ncols], F32R, tag="shift")
        nc.vector.tensor_mul(shift, mu, rstd)
        nc.vector.tensor_scalar_mul(shift, shift, -1.0)
        # broadcast to 128 partitions via K=1 matmul
        rstd_bc = ps_coef.tile([P, ncols], F32, tag="coef")
        shift_bc = ps_coef.tile([P, ncols], F32, tag="coef")
        _mm(nc, rstd_bc, ones_1x128, rstd, True, True)
        _mm(nc, shift_bc, ones_1x128, shift, True, True)
        if dst3 is not None:
            # one 3D op per pass; alternate the add between DVE and GpSimd so
            # neither engine serializes the block pipeline. GpSimd cannot read
            # PSUM, so stage the shift coefficients through SBUF for it.
            rb = rstd_bc.unsqueeze(1).to_broadcast(dst3.shape)
            if add_eng is nc.gpsimd:
                shift_sb = ctx_pool.tile([P, ncols], F32, tag="shift_sb",
                                         bufs=2)
                nc.scalar.copy(shift_sb, shift_bc)
                sb = shift_sb.unsqueeze(1).to_broadcast(dst3.shape)
            else:
                sb = shift_bc.unsqueeze(1).to_broadcast(dst3.shape)
            nc.vector.tensor_mul(dst3, src3, rb)
            add_eng.tensor_add(dst3, dst3, sb)
        else:
            for c in range(DC):
                nc.vector.tensor_mul(dst[:, c, :], src_tiles[c], rstd_bc)
                nc.vector.tensor_add(dst[:, c, :], dst[:, c, :], shift_bc)


    # persistent pools, allocated in reverse-release (stack) order
    vdram_pool = tc.alloc_tile_pool(name="vdram", bufs=1, space="DRAM")
    v_dram = vdram_pool.tile([KC, P, H, DH + 1], F32R)
    x2_pool = tc.alloc_tile_pool(name="x2p", bufs=1)
    x2acc = x2_pool.tile([P, DC, LQ], F32)
    x2 = x2_pool.tile([P, DC, LQ], F32R)
    kt_pool = tc.alloc_tile_pool(name="kt", bufs=1)
    k_t = kt_pool.tile([P, DC, L], F32R)  # [col-in-chunk, chunk, token]
    q_pool = tc.alloc_tile_pool(name="q", bufs=1)
    q_t = q_pool.tile([P, DC, LQ], F32R)
    normed_pool = tc.alloc_tile_pool(name="normed", bufs=1)
    normed_full = normed_pool.tile([P, DC, L], F32R)

    # =====================================================================
    # Phase A: LN1 over all tokens -> normed_full (feature-major, in place)
    # =====================================================================
    with (
        tc.tile_pool(name="ln1sq", bufs=2) as sq_pool,
        tc.tile_pool(name="ln1coef", bufs=1) as coef_small,
        tc.tile_pool(name="ps_stat", bufs=4, space="PSUM") as ps_stat,
        tc.tile_pool(name="ps_coef", bufs=2, space="PSUM") as ps_coef,
    ):
        for b in range(NBLK):
            blk = normed_full[:, :, b * BLK:(b + 1) * BLK]
            eng = nc.sync if b % 2 == 0 else nc.gpsimd
            eng.dma_start(blk, xt3[:, :, b * BLK:(b + 1) * BLK])
            layer_norm_t(coef_small, ps_stat, ps_coef,
                         [blk[:, c, :] for c in range(DC)], blk, BLK, sq_pool,
                         src3=blk, dst3=blk,
                         add_eng=nc.gpsimd if b % 2 == 0 else nc.vector)

    # =====================================================================
    # Phase C: own queries: LN1(own) -> q^T -> cosine-normalize * scaling
    # =====================================================================
    with (
        tc.tile_pool(name="qb", bufs=1) as qb_pool,
        tc.tile_pool(name="qsq", bufs=2) as qsq_pool,
        tc.tile_pool(name="qcoef", bufs=1) as qcoef,
        tc.tile_pool(name="wqstream", bufs=2) as wqstream,
    ):
        normed_own = qb_pool.tile([P, DC, LQ], F32R)
        nc.sync.dma_start(normed_own, xot3)
        with (
            tc.tile_pool(name="ps_stat2", bufs=2, space="PSUM") as ps_stat2,
            tc.tile_pool(name="ps_coef2", bufs=2, space="PSUM") as ps_coef2,
        ):
            layer_norm_t(qcoef, ps_stat2, ps_coef2,
                         [normed_own[:, c, :] for c in range(DC)], normed_own, LQ,
                         qsq_pool)
        with (
            tc.tile_pool(name="ps_mm2", bufs=2, space="PSUM") as ps_mm2,
            tc.tile_pool(name="ps_qn", bufs=2, space="PSUM") as ps_qn,
            tc.tile_pool(name="ps_qbc", bufs=2, space="PSUM") as ps_qbc,
        ):
            for m in range(DC):
                wqm = wqstream.tile([P, DC, P], F32R, tag="wq")
                nc.sync.dma_start(wqm, wq3[:, :, m * P:(m + 1) * P])
                ps = ps_mm2.tile([P, LQ], F32, tag="mm")
                for c in range(DC):
                    _mm(nc, ps, wqm[:, c, :], normed_own[:, c, :], c == 0,
                        c == DC - 1)
                nc.vector.tensor_scalar_add(q_t[:, m, :], ps, bq_sb[:, m:m + 1])
            # cosine-normalize q (x scaling folded into reciprocal)
            nsq = ps_qn.tile([H, LQ], F32, tag="qnsq")
            for m in range(DC):
                qsq = qsq_pool.tile([P, LQ], F32R, tag="xsq")
                nc.scalar.square(qsq, q_t[:, m, :])
                _mm(nc, nsq, selr_sb[:, m, :], qsq, m == 0, m == DC - 1)
            sd = qcoef.tile([H, LQ], F32, tag="qsd", bufs=2)
            nc.scalar.activation(sd, nsq,
                                 func=mybir.ActivationFunctionType.Sqrt,
                                 bias=0.0, scale=1.0)
            nc.vector.tensor_scalar_max(sd, sd, NORM_EPS)
            rec = qcoef.tile([H, LQ], F32R, tag="qrec", bufs=2)
            with nc.allow_low_precision(reason="f32r matmul operand"):
                nc.vector.reciprocal(rec, sd)
            nc.vector.tensor_scalar_mul(rec, rec, SCALING)
            for m in range(DC):
                bc = ps_qbc.tile([P, LQ], F32, tag="qbc")
                _mm(nc, bc, selb_sb[:, m, :], rec, True, True)
                nc.vector.tensor_mul(q_t[:, m, :], q_t[:, m, :], bc)

    # =====================================================================
    # Phase B: V (to DRAM scratch) then K^T + cosine-norm, block-pipelined
    # =====================================================================
    with (
        tc.tile_pool(name="wstream", bufs=2) as wstream,
        tc.tile_pool(name="vstage", bufs=3) as vstage,
        tc.tile_pool(name="knorm", bufs=2) as knorm_pool,
        tc.tile_pool(name="ps_mm", bufs=4, space="PSUM") as ps_mm,
        tc.tile_pool(name="ps_nrm", bufs=1, space="PSUM") as ps_nrm,
        tc.tile_pool(name="ps_nbc", bufs=1, space="PSUM") as ps_nbc,
    ):
        # V natural layout, block-major inside each quarter so the first
        # blocks of normed unblock V matmuls early
        QW = 256
        for n in range(4):
            wvn = wstream.tile([P, DC, QW], F32R, tag="wv")
            nc.gpsimd.dma_start(wvn, wv3[:, :, n * QW:(n + 1) * QW])
            for t in range(KC):
                ps = ps_mm.tile([P, QW], F32, tag="mmv", bufs=2)
                for c in range(DC):
                    _mm(nc, ps, normed_full[:, c, t * P:(t + 1) * P],
                        wvn[:, c, :], c == 0, c == DC - 1)
                stag = vstage.tile([P, 4, DH], F32R, tag="vstage")
                nc.vector.tensor_add(
                    stag, ps.rearrange("p (h d) -> p h d", d=DH),
                    bv_sb[:, n * QW:(n + 1) * QW].rearrange("p (h d) -> p h d",
                                                            d=DH))
                nc.gpsimd.dma_start(v_dram[t, :, n * 4:(n + 1) * 4, 0:DH], stag)
        # K block-outer with inline cosine-normalization, so attention's
        # exp work unblocks per block instead of all at the end
        for b in range(NBLK):
            for m in range(DC):
                wkm = wstream.tile([P, DC, P], F32R, tag="wk")
                nc.sync.dma_start(wkm, wk3[:, :, m * P:(m + 1) * P])
                ps = ps_mm.tile([P, BLK], F32, tag="mm")
                for c in range(DC):
                    _mm(nc, ps, wkm[:, c, :],
                        normed_full[:, c, b * BLK:(b + 1) * BLK], c == 0,
                        c == DC - 1)
                nc.vector.tensor_scalar_add(k_t[:, m, b * BLK:(b + 1) * BLK],
                                            ps, bk_sb[:, m:m + 1])
            nsq = ps_nrm.tile([H, BLK], F32, tag="nsq")
            for m in range(DC):
                ksq = knorm_pool.tile([P, BLK], F32R, tag="ksq")
                nc.scalar.square(ksq, k_t[:, m, b * BLK:(b + 1) * BLK])
                _mm(nc, nsq, selr_sb[:, m, :], ksq, m == 0, m == DC - 1)
            sd = knorm_pool.tile([H, BLK], F32, tag="ksd")
            nc.scalar.activation(sd, nsq,
                                 func=mybir.ActivationFunctionType.Sqrt,
                                 bias=0.0, scale=1.0)
            nc.vector.tensor_scalar_max(sd, sd, NORM_EPS)
            rec = knorm_pool.tile([H, BLK], F32R, tag="krec")
            with nc.allow_low_precision(reason="f32r matmul operand"):
                nc.vector.reciprocal(rec, sd)
            for m in range(DC):
                bc = ps_nbc.tile([P, BLK], F32, tag="nbc")
                _mm(nc, bc, selb_sb[:, m, :], rec, True, True)
                nc.vector.tensor_mul(k_t[:, m, b * BLK:(b + 1) * BLK],
                                     k_t[:, m, b * BLK:(b + 1) * BLK], bc)

    normed_pool.release()

    # =====================================================================
    # Phase D: attention per head-pair, with the out-projection folded in
    # (partial products accumulated into x2acc via DVE)
    # =====================================================================
    with (
        tc.tile_pool(name="exp", bufs=2) as exp_pool,
        tc.tile_pool(name="vsb", bufs=2) as vsb_pool,
        tc.tile_pool(name="rsc", bufs=2) as rsc_pool,
        tc.tile_pool(name="apair", bufs=2) as apair_pool,
        tc.tile_pool(name="wostream", bufs=2) as wostream,
        tc.tile_pool(name="ps_sc", bufs=2, space="PSUM") as ps_sc,
        tc.tile_pool(name="ps_acc", bufs=1, space="PSUM") as ps_acc,
        tc.tile_pool(name="ps_rbc", bufs=1, space="PSUM") as ps_rbc,
        tc.tile_pool(name="ps_op", bufs=2, space="PSUM") as ps_op,
    ):
        for m in range(DC):
            vp = vsb_pool.tile([P, KC, 2, DH + 1], F32R, tag="vp")
            for j in range(2):
                nc.gpsimd.dma_start(
                    vp[:, :, j, 0:DH],
                    v_dram[:, :, 2 * m + j, 0:DH].rearrange("k p d -> p k d"))
                nc.gpsimd.dma_start(
                    vp[:, :, j, DH:DH + 1],
                    vones_sb.rearrange("p (h o) -> p h o", o=1))
            eh = exp_pool.tile([P, KC, 2 * LQ], F32R, tag="exp")
            for kc in range(KC):
                # each head's scores go to a separate PSUM bank: fp32r matmul
                # writes at mid-bank free offsets fault on hardware
                ps = ps_sc.tile([P, 2, 2 * LQ], F32, tag="sc")
                for j in range(2):
                    _mm(nc, ps[:, j, 0:LQ],
                        k_t[j * DH:(j + 1) * DH, m, kc * P:(kc + 1) * P],
                        q_t[j * DH:(j + 1) * DH, m, :], True, True)
                nc.scalar.activation(
                    eh[:, kc, :].rearrange("p (j q) -> p j q", j=2),
                    ps[:, :, 0:LQ],
                    func=mybir.ActivationFunctionType.Exp,
                    bias=lcc_sb[:, kc:kc + 1], scale=1.0)
            attn_pair = apair_pool.tile([P, LQ], F32R, tag="apair")
            for j in range(2):
                acc = ps_acc.tile([DH + 1, LQ], F32, tag="acc")
                for kc in range(KC):
                    _mm(nc, acc, vp[:, kc, j, :],
                        eh[:, kc, j * LQ:(j + 1) * LQ], kc == 0, kc == KC - 1)
                recip = rsc_pool.tile([1, LQ], F32R, tag="recip")
                with nc.allow_low_precision(reason="f32r matmul operand"):
                    nc.vector.reciprocal(recip, acc[DH:DH + 1, :])
                rbc = ps_rbc.tile([DH, LQ], F32, tag="rbc")
                _mm(nc, rbc, ones_1x128[:, 0:DH], recip, True, True)
                rbc_sb = rsc_pool.tile([DH, LQ], F32, tag="rbcsb")
                nc.vector.tensor_copy(rbc_sb, rbc)
                nc.vector.tensor_mul(attn_pair[j * DH:(j + 1) * DH, :],
                                     acc[0:DH, :], rbc_sb)
            # out-projection partial for this pair-chunk of attn
            wom = wostream.tile([P, DC, P], F32R, tag="wo")
            nc.sync.dma_start(wom, wo3.rearrange("p c n -> p c n")[
                :, m, :].rearrange("p (o n) -> p o n", n=P))
            for o in range(DC):
                pso = ps_op.tile([P, LQ], F32, tag="op")
                _mm(nc, pso, wom[:, o, :], attn_pair, True, True)
                if m == 0:
                    nc.vector.tensor_copy(x2acc[:, o, :], pso)
                else:
                    nc.vector.tensor_add(x2acc[:, o, :], x2acc[:, o, :], pso)

    q_pool.release()
    kt_pool.release()

    # =====================================================================
    # Phase E: residual -> x2; LN2; FFN (ff2 single-pass, half-packed psum)
    # =====================================================================
    with (
        tc.tile_pool(name="xo2p", bufs=1) as xo2_pool,
        tc.tile_pool(name="ffsq", bufs=2) as ffsq_pool,
        tc.tile_pool(name="ffcoef", bufs=2) as ffcoef,
        tc.tile_pool(name="ht", bufs=1) as ht_pool,
        tc.tile_pool(name="wf1s", bufs=3) as wf1s,
        tc.tile_pool(name="wf2s", bufs=3) as wf2s,
        tc.tile_pool(name="outsb", bufs=2) as outsb_pool,
    ):
        xo2 = xo2_pool.tile([P, DC, LQ], F32R)
        nc.sync.dma_start(xo2, xot3)
        for o in range(DC):
            nc.vector.tensor_scalar_add(x2[:, o, :], x2acc[:, o, :],
                                        bo_sb[:, o:o + 1])
            nc.vector.tensor_add(x2[:, o, :], x2[:, o, :], xo2[:, o, :])
        normed2 = xo2_pool.tile([P, DC, LQ], F32R)
        with (
            tc.tile_pool(name="ps_stat3", bufs=2, space="PSUM") as ps_stat3,
            tc.tile_pool(name="ps_coef3", bufs=2, space="PSUM") as ps_coef3,
        ):
            layer_norm_t(ffcoef, ps_stat3, ps_coef3,
                         [x2[:, c, :] for c in range(DC)], normed2, LQ,
                         ffsq_pool)
        ps_mm3 = tc.alloc_tile_pool(name="ps_mm3", bufs=3, space="PSUM")
        ps_ff2 = tc.alloc_tile_pool(name="ps_ff2", bufs=4, space="PSUM")
        h_t = ht_pool.tile([P, FC, LQ], F32R)
        wf24 = wf23.rearrange("p c (g n) -> p c g n", g=2)  # [128,32,2,512]
        for f in range(FC):
            wf1m = wf1s.tile([P, DC, P], F32R, tag="wf1")
            weng = nc.sync if f % 2 == 0 else nc.gpsimd
            weng.dma_start(wf1m, wf13[:, :, f * P:(f + 1) * P])
            ps = ps_mm3.tile([P, LQ], F32, tag="mm")
            for c in range(DC):
                _mm(nc, ps, wf1m[:, c, :], normed2[:, c, :], c == 0, c == DC - 1)
            nc.scalar.activation(h_t[:, f, :], ps, func=GELU_FUNC,
                                 bias=bf1_sb[:, f:f + 1], scale=1.0)
        # ff2: f-outer accumulation in two 4-output passes; pass 1 pipelines
        # with ff1 chunk by chunk
        for g in range(2):
            accs = [ps_ff2.tile([P, LQ], F32, tag="ff2acc",
                                name=f"ff2acc_{g}_{i}") for i in range(4)]
            for f in range(FC):
                wf2m = wf2s.tile([P, 4, P], F32R, tag="wf2")
                weng2 = nc.gpsimd if f % 2 == 0 else nc.sync
                weng2.dma_start(wf2m, wf24[:, f, g, :].rearrange(
                    "p (i n) -> p i n", n=P))
                for i in range(4):
                    _mm(nc, accs[i], wf2m[:, i, :], h_t[:, f, :],
                        f == 0, f == FC - 1)
            for i in range(4):
                mcol = g * 4 + i
                osb = outsb_pool.tile([P, LQ], F32, tag="osb")
                nc.vector.tensor_scalar_add(osb, accs[i], bf2_sb[:, mcol:mcol + 1])
                nc.vector.tensor_add(osb, osb, x2[:, mcol, :])
                nc.sync.dma_start(out3[:, mcol, :], osb)
        ps_ff2.release()
        ps_mm3.release()

    x2_pool.release()
    vdram_pool.release()
    singles.release()


_CACHED = None


def build():
    global _CACHED
    if _CACHED is None:
        nc = bacc.Bacc("TRN2", target_bir_lowering=False, debug=False)
        with tile.TileContext(nc) as tc:
            emit(tc)
        nc.compile()
        _CACHED = nc
    return _CACHED


def _onesc_matrix():
    o = np.zeros((P, 3), np.float32)
    o[:, 0] = 1.0
    o[0:DH, 1] = 1.0
    o[DH:P, 2] = 1.0
    return o


def _selr_matrix():
    # [P, DC*H]: selr[p, m*16+h] = 1 iff h == 2m + (p >= 64)
    s = np.zeros((P, DC, H), np.float32)
    for m in range(DC):
        s[0:DH, m, 2 * m] = 1.0
        s[DH:P, m, 2 * m + 1] = 1.0
    return np.ascontiguousarray(s.reshape(P, P))


def _selb_matrix():
    # [H, DC*P]: selb[h, m*128+p] = 1 iff h == 2m + (p >= 64)
    s = np.zeros((H, DC, P), np.float32)
    for m in range(DC):
        s[2 * m, m, 0:DH] = 1.0
        s[2 * m + 1, m, DH:P] = 1.0
    return np.ascontiguousarray(s.reshape(H, DC * P))


def prep_inputs(inputs):
    """Host-side preprocessing: transpose x, split/fold weights, bias layouts."""
    f = np.float32
    x = np.asarray(inputs["x"], f)
    lcc = np.asarray(inputs["lcc_values"], f)
    w_qkv = np.asarray(inputs["w_qkv"], f)
    b_qkv = np.asarray(inputs["b_qkv"], f)
    ln1_g = np.asarray(inputs["ln1_g"], f)
    ln1_b = np.asarray(inputs["ln1_b"], f)
    ln2_g = np.asarray(inputs["ln2_g"], f)
    ln2_b = np.asarray(inputs["ln2_b"], f)
    w_ff1 = np.asarray(inputs["w_ff1"], f)
    b_ff1 = np.asarray(inputs["b_ff1"], f)

    def chunked(b):  # [D] -> [128, DC] with chunk c in column c
        return np.ascontiguousarray(b.reshape(-1, P).T)

    xt = np.ascontiguousarray(x.T)
    shared = {
        "xt": xt,
        "wq": np.ascontiguousarray(ln1_g[:, None] * w_qkv[:, 0:D]),
        "wk": np.ascontiguousarray(ln1_g[:, None] * w_qkv[:, D:2 * D]),
        "wv": np.ascontiguousarray(ln1_g[:, None] * w_qkv[:, 2 * D:3 * D]),
        "wo": np.ascontiguousarray(np.asarray(inputs["w_out"], f)),
        "wf1": np.ascontiguousarray(ln2_g[:, None] * w_ff1),
        "wf2": np.ascontiguousarray(np.asarray(inputs["w_ff2"], f)),
        "bq": chunked(b_qkv[0:D] + ln1_b @ w_qkv[:, 0:D]),
        "bk": chunked(b_qkv[D:2 * D] + ln1_b @ w_qkv[:, D:2 * D]),
        "bv": np.ascontiguousarray(b_qkv[2 * D:3 * D] + ln1_b @ w_qkv[:, 2 * D:3 * D]),
        "bo": chunked(np.asarray(inputs["b_out"], f)),
        "bf1": chunked(b_ff1 + ln2_b @ w_ff1),
        "bf2": chunked(np.asarray(inputs["b_ff2"], f)),
        "lcck": np.ascontiguousarray((lcc * (0.5 * LCC)).reshape(KC, P).T),
        "selr": _selr_matrix(),
        "selb": _selb_matrix(),
        "onesc": _onesc_matrix(),
        "ones1r": np.ones((1, P), np.float32),
        "vones": np.ones((P, KC), np.float32),
    }
    in_maps = []
    for c in range(NCORES):
        m = dict(shared)
        m["xot"] = np.ascontiguousarray(xt[:, c * LQ:(c + 1) * LQ])
        in_maps.append(m)
    return in_maps


def kernel(**inputs):
    nc = build()
    in_maps = prep_inputs(inputs)
    res = run_bass_kernel_spmd(nc, in_maps, core_ids=list(range(NCORES)))
    out = np.concatenate([res.results[c]["out_t"] for c in range(NCORES)], axis=1)
    return np.ascontiguousarray(out.T).astype(np.float32)



# revision 5
# speedup vs baseline: 1.1042x; 1.1042x over previous
"""Trainium2 Bass kernel for EnhancedMultiHeadSelfAttention (dense transformer block).

Sharding: sequence-parallel over 8 cores. Each core owns L/8 = 256 query rows.
LN1 + K/V projection for all 2048 tokens are replicated on every core (cheaper
than on-chip AllReduce at this size); scores/softmax/attn@V/out-proj/LN2/FFN are
computed only for the core's own 256 rows. No collectives.

Layout: activations are kept feature-major ("transposed", [feature, token]) so
every linear layer is matmul(out=[cols, tok], lhsT=W[k,cols], rhs=actT[k,tok])
with natural weight layout and no on-device transposes. All matmuls run as
float32r (full fp32 data, bf16-rate PE throughput for free dim >= 256).

Math notes:
 - clip(scores,-10,10) never binds: |cos|*0.125 + bias in [-0.125, 0.225].
 - softmax needs no max-subtraction for the same reason.
 - the query-side half of the lcc bias is a per-query constant factor in
   exp-space and cancels in softmax normalization; only the key-side half is
   applied (as per-partition ACT bias in the exp).
 - softmax denominators come from an appended ones-column in V.
 - LN gains/biases are folded into the following matmul's weights on the host.
"""

import numpy as np

import concourse.bass as bass
import concourse.tile as tile
from concourse import bacc, mybir
from concourse.bass_utils import run_bass_kernel_spmd

F32 = mybir.dt.float32
F32R = mybir.dt.float32r

L = 2048          # sequence length
D = 1024          # model dim
H = 16            # heads
DH = 64           # head dim
FF = 4096         # ffn hidden
P = 128           # partitions
NCORES = 8
LQ = L // NCORES  # 256 own query rows per core
DC = D // P       # 8 d-model chunks
FC = FF // P      # 32 ffn chunks
KC = L // P       # 16 key chunks
NBLK = 4          # token blocks of 512 for the replicated phase
BLK = L // NBLK   # 512

# CoreSim doesn't implement Gelu; test_sim swaps this to Identity and checks
# against a gelu-less reference. Hardware always uses the real (erf) Gelu.
GELU_FUNC = mybir.ActivationFunctionType.Gelu

LN_EPS = 1e-5
NORM_EPS = 1e-12
SCALING = DH ** -0.5
LCC = 0.1


def _mm(nc, out, lhsT, rhs, start, stop):
    assert lhsT.dtype == F32R and rhs.dtype == F32R, (lhsT.dtype, rhs.dtype)
    nc.tensor.matmul(out, lhsT, rhs, start=start, stop=stop)


# ---- packed-input layout ---------------------------------------------------
# All ExternalInputs are packed into two flat DRAM tensors (pack_r: f32r
# matmul operands, pack_f: f32 bias/coef tensors). The axon client pays a
# fixed per-buffer enqueue cost (~30us) per execution, so 21 NEFF inputs
# cost ~0.7us more per exec than 2. Each segment is stored host-side
# C-contiguous in exactly the [partition, chunk, col] view shape the body
# uses, so views are simple strided APs.

_R_SEGS = [
    ("xt3", (P, DC, L)),
    ("xot3", (P, DC, LQ)),
    ("wq3", (P, DC, D)),
    ("wk3", (P, DC, D)),
    ("wv3", (P, DC, D)),
    ("wo3", (P, DC, D)),
    ("wf13", (P, DC, FF)),
    ("wf23", (P, FC, D)),
    ("selr3", (P, DC, H)),
    ("selb3", (H, DC, P)),
    ("onesc", (P, 3)),
    ("ones1r", (1, P)),
    ("vones", (P, KC)),
]
_F_SEGS = [
    ("bq", (P, DC)),
    ("bk", (P, DC)),
    ("bv", (D,)),
    ("bo", (P, DC)),
    ("bf1", (P, FC)),
    ("bf2", (P, DC)),
    ("lcck", (P, KC)),
]


def _seg_offsets(segs):
    offs, acc = {}, 0
    for name, shape in segs:
        offs[name] = acc
        acc += int(np.prod(shape))
    return offs, acc


_R_OFFS, _R_TOT = _seg_offsets(_R_SEGS)
_F_OFFS, _F_TOT = _seg_offsets(_F_SEGS)


def _view(pack_ap, off, shape):
    strides, acc = [], 1
    for s in reversed(shape):
        strides.append(acc)
        acc *= s
    strides = list(reversed(strides))
    return bass.AP(tensor=pack_ap.tensor, offset=off,
                   ap=[[st, sz] for st, sz in zip(strides, shape)])


def declare_io(nc):
    pack_r = nc.dram_tensor("pack_r", [_R_TOT], F32R, kind="ExternalInput").ap()
    pack_f = nc.dram_tensor("pack_f", [_F_TOT], F32, kind="ExternalInput").ap()
    out_t = nc.dram_tensor("out_t", [D, LQ], F32, kind="ExternalOutput").ap()

    io = {}
    for name, shape in _R_SEGS:
        io[name] = _view(pack_r, _R_OFFS[name], shape)
    for name, shape in _F_SEGS:
        io[name] = _view(pack_f, _F_OFFS[name], shape)
    # b_v broadcast to all partitions: stride-0 partition dim
    io["bv_bc"] = bass.AP(tensor=pack_f.tensor, offset=_F_OFFS["bv"],
                          ap=[[0, P], [1, D]])
    io["out3"] = out_t.rearrange("(c p) t -> p c t", p=P)  # [128, 8, 256]
    io["out_t"] = out_t
    return io


def emit(tc):
    body(tc, declare_io(tc.nc))


def body(tc, io, sfx=""):
    nc = tc.nc

    xt3 = io["xt3"]          # [128, 8, 2048]
    xot3 = io["xot3"]        # [128, 8, 256]
    wq3 = io["wq3"]          # [128, 8, 1024]
    wk3 = io["wk3"]
    wv3 = io["wv3"]
    wo3 = io["wo3"]
    wf13 = io["wf13"]        # [128, 8, 4096]
    wf23 = io["wf23"]        # [128, 32, 1024]
    out3 = io["out3"]        # [128, 8, 256]

    # ---- persistent small constants -------------------------------------
    singles = tc.alloc_tile_pool(name="singles" + sfx, bufs=1)
    ones_1x128 = singles.tile([1, P], F32R)  # K=1 broadcast lhsT
    nc.sync.dma_start(ones_1x128, io["ones1r"])
    onesc_sb = singles.tile([P, 3], F32R)
    nc.sync.dma_start(onesc_sb, io["onesc"])
    ones_col = onesc_sb[:, 0:1]              # K=128 -> M=1 reduction lhsT
    # head-norm selectors (host-precomputed):
    # selr_sb[:, m, h] = 1 if head h belongs to chunk m at this partition;
    # selb_sb[h, m, p] = transpose, for broadcasting norms back to chunks
    selr_sb = singles.tile([P, DC, H], F32R)
    nc.sync.dma_start(selr_sb, io["selr3"])
    selb_sb = singles.tile([H, DC, P], F32R)
    nc.sync.dma_start(selb_sb, io["selb3"])
    vones_sb = singles.tile([P, KC], F32R)
    nc.sync.dma_start(vones_sb, io["vones"])
    bq_sb = singles.tile([P, DC], F32)
    nc.sync.dma_start(bq_sb, io["bq"])
    bk_sb = singles.tile([P, DC], F32)
    nc.sync.dma_start(bk_sb, io["bk"])
    bo_sb = singles.tile([P, DC], F32)
    nc.sync.dma_start(bo_sb, io["bo"])
    bf1_sb = singles.tile([P, FC], F32)
    nc.sync.dma_start(bf1_sb, io["bf1"])
    bf2_sb = singles.tile([P, DC], F32)
    nc.sync.dma_start(bf2_sb, io["bf2"])
    lcc_sb = singles.tile([P, KC], F32)
    nc.sync.dma_start(lcc_sb, io["lcck"])
    bv_sb = singles.tile([P, D], F32)  # b_v broadcast to all partitions
    nc.sync.dma_start(bv_sb, io["bv_bc"])
    eps_sb = singles.tile([1, 1], F32)
    nc.vector.memset(eps_sb, LN_EPS)

    def layer_norm_t(ctx_pool, ps_stat, ps_coef, src_tiles, dst, ncols, sq_pool,
                     src3=None, dst3=None, add_eng=None):
        """LayerNorm along feature dim for feature-major tiles.

        src_tiles: list of DC tiles/APs [128, ncols] (feature chunks)
        dst: [128, DC, ncols] output tile
        """
        sums = ps_stat.tile([1, ncols], F32, tag="stat")
        sumsq = ps_stat.tile([1, ncols], F32, tag="stat")
        for c in range(DC):
            xc = src_tiles[c]
            xsq = sq_pool.tile([P, ncols], F32R, tag="xsq")
            nc.scalar.square(xsq, xc)
            _mm(nc, sums, ones_col, xc, c == 0, c == DC - 1)
            _mm(nc, sumsq, ones_col, xsq, c == 0, c == DC - 1)
        # coeffs on one partition: rstd, shift = -mu*rstd
        mu = ctx_pool.tile([1, ncols], F32, tag="mu")
        nc.vector.tensor_scalar_mul(mu, sums, 1.0 / D)
        ex2 = ctx_pool.tile([1, ncols], F32, tag="ex2")
        nc.vector.tensor_scalar_mul(ex2, sumsq, 1.0 / D)
        var = ctx_pool.tile([1, ncols], F32, tag="var")
        nc.vector.tensor_mul(var, mu, mu)
        nc.vector.tensor_sub(var, ex2, var)
        sd = ctx_pool.tile([1, ncols], F32, tag="sd")
        nc.scalar.activation(sd, var, func=mybir.ActivationFunctionType.Sqrt,
                             bias=eps_sb, scale=1.0)
        rstd = ctx_pool.tile([1, ncols], F32R, tag="rstd")
        with nc.allow_low_precision(reason="f32r matmul operand"):
            nc.vector.reciprocal(rstd, sd)
        shift = ctx_pool.tile([1, ncols], F32R, tag="shift")
        nc.vector.tensor_mul(shift, mu, rstd)
        nc.vector.tensor_scalar_mul(shift, shift, -1.0)
        # broadcast to 128 partitions via K=1 matmul
        rstd_bc = ps_coef.tile([P, ncols], F32, tag="coef")
        shift_bc = ps_coef.tile([P, ncols], F32, tag="coef")
        _mm(nc, rstd_bc, ones_1x128, rstd, True, True)
        _mm(nc, shift_bc, ones_1x128, shift, True, True)
        if dst3 is not None:
            # one 3D op per pass; alternate the add between DVE and GpSimd so
            # neither engine serializes the block pipeline. GpSimd cannot read
            # PSUM, so stage the shift coefficients through SBUF for it.
            rb = rstd_bc.unsqueeze(1).to_broadcast(dst3.shape)
            if add_eng is nc.gpsimd:
                shift_sb = ctx_pool.tile([P, ncols], F32, tag="shift_sb",
                                         bufs=2)
                nc.scalar.copy(shift_sb, shift_bc)
                sb = shift_sb.unsqueeze(1).to_broadcast(dst3.shape)
            else:
                sb = shift_bc.unsqueeze(1).to_broadcast(dst3.shape)
            nc.vector.tensor_mul(dst3, src3, rb)
            add_eng.tensor_add(dst3, dst3, sb)
        else:
            for c in range(DC):
                nc.vector.tensor_mul(dst[:, c, :], src_tiles[c], rstd_bc)
                nc.vector.tensor_add(dst[:, c, :], dst[:, c, :], shift_bc)


    # persistent pools, allocated in reverse-release (stack) order
    vdram_pool = tc.alloc_tile_pool(name="vdram", bufs=1, space="DRAM")
    v_dram = vdram_pool.tile([KC, P, H, DH + 1], F32R)
    x2_pool = tc.alloc_tile_pool(name="x2p", bufs=1)
    x2acc = x2_pool.tile([P, DC, LQ], F32)
    x2 = x2_pool.tile([P, DC, LQ], F32R)
    kt_pool = tc.alloc_tile_pool(name="kt", bufs=1)
    k_t = kt_pool.tile([P, DC, L], F32R)  # [col-in-chunk, chunk, token]
    q_pool = tc.alloc_tile_pool(name="q", bufs=1)
    q_t = q_pool.tile([P, DC, LQ], F32R)
    normed_pool = tc.alloc_tile_pool(name="normed", bufs=1)
    normed_full = normed_pool.tile([P, DC, L], F32R)

    # =====================================================================
    # Phase A: LN1 over all tokens -> normed_full (feature-major, in place)
    # =====================================================================
    with (
        tc.tile_pool(name="ln1sq", bufs=2) as sq_pool,
        tc.tile_pool(name="ln1coef", bufs=1) as coef_small,
        tc.tile_pool(name="ps_stat", bufs=4, space="PSUM") as ps_stat,
        tc.tile_pool(name="ps_coef", bufs=2, space="PSUM") as ps_coef,
    ):
        for b in range(NBLK):
            blk = normed_full[:, :, b * BLK:(b + 1) * BLK]
            eng = nc.sync if b % 2 == 0 else nc.gpsimd
            eng.dma_start(blk, xt3[:, :, b * BLK:(b + 1) * BLK])
            layer_norm_t(coef_small, ps_stat, ps_coef,
                         [blk[:, c, :] for c in range(DC)], blk, BLK, sq_pool,
                         src3=blk, dst3=blk,
                         add_eng=nc.gpsimd if b % 2 == 0 else nc.vector)

    # =====================================================================
    # Phase C: own queries: LN1(own) -> q^T -> cosine-normalize * scaling
    # =====================================================================
    with (
        tc.tile_pool(name="qb", bufs=1) as qb_pool,
        tc.tile_pool(name="qsq", bufs=2) as qsq_pool,
        tc.tile_pool(name="qcoef", bufs=1) as qcoef,
        tc.tile_pool(name="wqstream", bufs=2) as wqstream,
    ):
        normed_own = qb_pool.tile([P, DC, LQ], F32R)
        nc.sync.dma_start(normed_own, xot3)
        with (
            tc.tile_pool(name="ps_stat2", bufs=2, space="PSUM") as ps_stat2,
            tc.tile_pool(name="ps_coef2", bufs=2, space="PSUM") as ps_coef2,
        ):
            layer_norm_t(qcoef, ps_stat2, ps_coef2,
                         [normed_own[:, c, :] for c in range(DC)], normed_own, LQ,
                         qsq_pool)
        with (
            tc.tile_pool(name="ps_mm2", bufs=2, space="PSUM") as ps_mm2,
            tc.tile_pool(name="ps_qn", bufs=2, space="PSUM") as ps_qn,
            tc.tile_pool(name="ps_qbc", bufs=2, space="PSUM") as ps_qbc,
        ):
            for m in range(DC):
                wqm = wqstream.tile([P, DC, P], F32R, tag="wq")
                nc.sync.dma_start(wqm, wq3[:, :, m * P:(m + 1) * P])
                ps = ps_mm2.tile([P, LQ], F32, tag="mm")
                for c in range(DC):
                    _mm(nc, ps, wqm[:, c, :], normed_own[:, c, :], c == 0,
                        c == DC - 1)
                nc.vector.tensor_scalar_add(q_t[:, m, :], ps, bq_sb[:, m:m + 1])
            # cosine-normalize q (x scaling folded into reciprocal)
            nsq = ps_qn.tile([H, LQ], F32, tag="qnsq")
            for m in range(DC):
                qsq = qsq_pool.tile([P, LQ], F32R, tag="xsq")
                nc.scalar.square(qsq, q_t[:, m, :])
                _mm(nc, nsq, selr_sb[:, m, :], qsq, m == 0, m == DC - 1)
            sd = qcoef.tile([H, LQ], F32, tag="qsd", bufs=2)
            nc.scalar.activation(sd, nsq,
                                 func=mybir.ActivationFunctionType.Sqrt,
                                 bias=0.0, scale=1.0)
            nc.vector.tensor_scalar_max(sd, sd, NORM_EPS)
            rec = qcoef.tile([H, LQ], F32R, tag="qrec", bufs=2)
            with nc.allow_low_precision(reason="f32r matmul operand"):
                nc.vector.reciprocal(rec, sd)
            nc.vector.tensor_scalar_mul(rec, rec, SCALING)
            for m in range(DC):
                bc = ps_qbc.tile([P, LQ], F32, tag="qbc")
                _mm(nc, bc, selb_sb[:, m, :], rec, True, True)
                nc.vector.tensor_mul(q_t[:, m, :], q_t[:, m, :], bc)

    # =====================================================================
    # Phase B: V (to DRAM scratch) then K^T + cosine-norm, block-pipelined
    # =====================================================================
    with (
        tc.tile_pool(name="wstream", bufs=2) as wstream,
        tc.tile_pool(name="vstage", bufs=3) as vstage,
        tc.tile_pool(name="knorm", bufs=2) as knorm_pool,
        tc.tile_pool(name="ps_mm", bufs=4, space="PSUM") as ps_mm,
        tc.tile_pool(name="ps_nrm", bufs=1, space="PSUM") as ps_nrm,
        tc.tile_pool(name="ps_nbc", bufs=1, space="PSUM") as ps_nbc,
    ):
        # V natural layout, block-major inside each quarter so the first
        # blocks of normed unblock V matmuls early
        QW = 256
        for n in range(4):
            wvn = wstream.tile([P, DC, QW], F32R, tag="wv")
            nc.gpsimd.dma_start(wvn, wv3[:, :, n * QW:(n + 1) * QW])
            for t in range(KC):
                ps = ps_mm.tile([P, QW], F32, tag="mmv", bufs=2)
                for c in range(DC):
                    _mm(nc, ps, normed_full[:, c, t * P:(t + 1) * P],
                        wvn[:, c, :], c == 0, c == DC - 1)
                stag = vstage.tile([P, 4, DH], F32R, tag="vstage")
                nc.vector.tensor_add(
                    stag, ps.rearrange("p (h d) -> p h d", d=DH),
                    bv_sb[:, n * QW:(n + 1) * QW].rearrange("p (h d) -> p h d",
                                                            d=DH))
                nc.gpsimd.dma_start(v_dram[t, :, n * 4:(n + 1) * 4, 0:DH], stag)
        # K block-outer with inline cosine-normalization, so attention's
        # exp work unblocks per block instead of all at the end
        for b in range(NBLK):
            for m in range(DC):
                wkm = wstream.tile([P, DC, P], F32R, tag="wk")
                nc.sync.dma_start(wkm, wk3[:, :, m * P:(m + 1) * P])
                ps = ps_mm.tile([P, BLK], F32, tag="mm")
                for c in range(DC):
                    _mm(nc, ps, wkm[:, c, :],
                        normed_full[:, c, b * BLK:(b + 1) * BLK], c == 0,
                        c == DC - 1)
                nc.vector.tensor_scalar_add(k_t[:, m, b * BLK:(b + 1) * BLK],
                                            ps, bk_sb[:, m:m + 1])
            nsq = ps_nrm.tile([H, BLK], F32, tag="nsq")
            for m in range(DC):
                ksq = knorm_pool.tile([P, BLK], F32R, tag="ksq")
                nc.scalar.square(ksq, k_t[:, m, b * BLK:(b + 1) * BLK])
                _mm(nc, nsq, selr_sb[:, m, :], ksq, m == 0, m == DC - 1)
            sd = knorm_pool.tile([H, BLK], F32, tag="ksd")
            nc.scalar.activation(sd, nsq,
                                 func=mybir.ActivationFunctionType.Sqrt,
                                 bias=0.0, scale=1.0)
            nc.vector.tensor_scalar_max(sd, sd, NORM_EPS)
            rec = knorm_pool.tile([H, BLK], F32R, tag="krec")
            with nc.allow_low_precision(reason="f32r matmul operand"):
                nc.vector.reciprocal(rec, sd)
            for m in range(DC):
                bc = ps_nbc.tile([P, BLK], F32, tag="nbc")
                _mm(nc, bc, selb_sb[:, m, :], rec, True, True)
                nc.vector.tensor_mul(k_t[:, m, b * BLK:(b + 1) * BLK],
                                     k_t[:, m, b * BLK:(b + 1) * BLK], bc)

    normed_pool.release()

    # =====================================================================
    # Phase D: attention per head-pair, with the out-projection folded in
    # (partial products accumulated into x2acc via DVE)
    # =====================================================================
    with (
        tc.tile_pool(name="exp", bufs=2) as exp_pool,
        tc.tile_pool(name="vsb", bufs=2) as vsb_pool,
        tc.tile_pool(name="rsc", bufs=2) as rsc_pool,
        tc.tile_pool(name="apair", bufs=2) as apair_pool,
        tc.tile_pool(name="wostream", bufs=2) as wostream,
        tc.tile_pool(name="ps_sc", bufs=2, space="PSUM") as ps_sc,
        tc.tile_pool(name="ps_acc", bufs=1, space="PSUM") as ps_acc,
        tc.tile_pool(name="ps_rbc", bufs=1, space="PSUM") as ps_rbc,
        tc.tile_pool(name="ps_op", bufs=2, space="PSUM") as ps_op,
    ):
        for m in range(DC):
            vp = vsb_pool.tile([P, KC, 2, DH + 1], F32R, tag="vp")
            for j in range(2):
                nc.gpsimd.dma_start(
                    vp[:, :, j, 0:DH],
                    v_dram[:, :, 2 * m + j, 0:DH].rearrange("k p d -> p k d"))
                nc.gpsimd.dma_start(
                    vp[:, :, j, DH:DH + 1],
                    vones_sb.rearrange("p (h o) -> p h o", o=1))
            eh = exp_pool.tile([P, KC, 2 * LQ], F32R, tag="exp")
            for kc in range(KC):
                # each head's scores go to a separate PSUM bank: fp32r matmul
                # writes at mid-bank free offsets fault on hardware
                ps = ps_sc.tile([P, 2, 2 * LQ], F32, tag="sc")
                for j in range(2):
                    _mm(nc, ps[:, j, 0:LQ],
                        k_t[j * DH:(j + 1) * DH, m, kc * P:(kc + 1) * P],
                        q_t[j * DH:(j + 1) * DH, m, :], True, True)
                nc.scalar.activation(
                    eh[:, kc, :].rearrange("p (j q) -> p j q", j=2),
                    ps[:, :, 0:LQ],
                    func=mybir.ActivationFunctionType.Exp,
                    bias=lcc_sb[:, kc:kc + 1], scale=1.0)
            attn_pair = apair_pool.tile([P, LQ], F32R, tag="apair")
            for j in range(2):
                acc = ps_acc.tile([DH + 1, LQ], F32, tag="acc")
                for kc in range(KC):
                    _mm(nc, acc, vp[:, kc, j, :],
                        eh[:, kc, j * LQ:(j + 1) * LQ], kc == 0, kc == KC - 1)
                recip = rsc_pool.tile([1, LQ], F32R, tag="recip")
                with nc.allow_low_precision(reason="f32r matmul operand"):
                    nc.vector.reciprocal(recip, acc[DH:DH + 1, :])
                rbc = ps_rbc.tile([DH, LQ], F32, tag="rbc")
                _mm(nc, rbc, ones_1x128[:, 0:DH], recip, True, True)
                rbc_sb = rsc_pool.tile([DH, LQ], F32, tag="rbcsb")
                nc.vector.tensor_copy(rbc_sb, rbc)
                nc.vector.tensor_mul(attn_pair[j * DH:(j + 1) * DH, :],
                                     acc[0:DH, :], rbc_sb)
            # out-projection partial for this pair-chunk of attn
            wom = wostream.tile([P, DC, P], F32R, tag="wo")
            nc.sync.dma_start(wom, wo3.rearrange("p c n -> p c n")[
                :, m, :].rearrange("p (o n) -> p o n", n=P))
            for o in range(DC):
                pso = ps_op.tile([P, LQ], F32, tag="op")
                _mm(nc, pso, wom[:, o, :], attn_pair, True, True)
                if m == 0:
                    nc.vector.tensor_copy(x2acc[:, o, :], pso)
                else:
                    nc.vector.tensor_add(x2acc[:, o, :], x2acc[:, o, :], pso)

    q_pool.release()
    kt_pool.release()

    # =====================================================================
    # Phase E: residual -> x2; LN2; FFN (ff2 single-pass, half-packed psum)
    # =====================================================================
    with (
        tc.tile_pool(name="xo2p", bufs=1) as xo2_pool,
        tc.tile_pool(name="ffsq", bufs=2) as ffsq_pool,
        tc.tile_pool(name="ffcoef", bufs=2) as ffcoef,
        tc.tile_pool(name="ht", bufs=1) as ht_pool,
        tc.tile_pool(name="wf1s", bufs=3) as wf1s,
        tc.tile_pool(name="wf2s", bufs=3) as wf2s,
        tc.tile_pool(name="outsb", bufs=2) as outsb_pool,
    ):
        xo2 = xo2_pool.tile([P, DC, LQ], F32R)
        nc.sync.dma_start(xo2, xot3)
        for o in range(DC):
            nc.vector.tensor_scalar_add(x2[:, o, :], x2acc[:, o, :],
                                        bo_sb[:, o:o + 1])
            nc.vector.tensor_add(x2[:, o, :], x2[:, o, :], xo2[:, o, :])
        normed2 = xo2_pool.tile([P, DC, LQ], F32R)
        with (
            tc.tile_pool(name="ps_stat3", bufs=2, space="PSUM") as ps_stat3,
            tc.tile_pool(name="ps_coef3", bufs=2, space="PSUM") as ps_coef3,
        ):
            layer_norm_t(ffcoef, ps_stat3, ps_coef3,
                         [x2[:, c, :] for c in range(DC)], normed2, LQ,
                         ffsq_pool)
        ps_mm3 = tc.alloc_tile_pool(name="ps_mm3", bufs=3, space="PSUM")
        ps_ff2 = tc.alloc_tile_pool(name="ps_ff2", bufs=4, space="PSUM")
        h_t = ht_pool.tile([P, FC, LQ], F32R)
        wf24 = wf23.rearrange("p c (g n) -> p c g n", g=2)  # [128,32,2,512]
        for f in range(FC):
            wf1m = wf1s.tile([P, DC, P], F32R, tag="wf1")
            weng = nc.sync if f % 2 == 0 else nc.gpsimd
            weng.dma_start(wf1m, wf13[:, :, f * P:(f + 1) * P])
            ps = ps_mm3.tile([P, LQ], F32, tag="mm")
            for c in range(DC):
                _mm(nc, ps, wf1m[:, c, :], normed2[:, c, :], c == 0, c == DC - 1)
            nc.scalar.activation(h_t[:, f, :], ps, func=GELU_FUNC,
                                 bias=bf1_sb[:, f:f + 1], scale=1.0)
        # ff2: f-outer accumulation in two 4-output passes; pass 1 pipelines
        # with ff1 chunk by chunk
        for g in range(2):
            accs = [ps_ff2.tile([P, LQ], F32, tag="ff2acc",
                                name=f"ff2acc_{g}_{i}") for i in range(4)]
            for f in range(FC):
                wf2m = wf2s.tile([P, 4, P], F32R, tag="wf2")
                weng2 = nc.gpsimd if f % 2 == 0 else nc.sync
                weng2.dma_start(wf2m, wf24[:, f, g, :].rearrange(
                    "p (i n) -> p i n", n=P))
                for i in range(4):
                    _mm(nc, accs[i], wf2m[:, i, :], h_t[:, f, :],
                        f == 0, f == FC - 1)
            for i in range(4):
                mcol = g * 4 + i
                osb = outsb_pool.tile([P, LQ], F32, tag="osb")
                nc.vector.tensor_scalar_add(osb, accs[i], bf2_sb[:, mcol:mcol + 1])
                nc.vector.tensor_add(osb, osb, x2[:, mcol, :])
                nc.sync.dma_start(out3[:, mcol, :], osb)
        ps_ff2.release()
        ps_mm3.release()

    x2_pool.release()
    vdram_pool.release()
    singles.release()


_CACHED = {}


def build_k(k=1):
    """Build a NEFF with the kernel body emitted k times back-to-back.

    k=1 is the production kernel; k>1 exists so a timing harness can
    measure steady-state per-execution time on-device (the repetitions
    are data-independent re-runs writing the same output)."""
    if k not in _CACHED:
        nc = bacc.Bacc("TRN2", target_bir_lowering=False, debug=False)
        with tile.TileContext(nc) as tc:
            io = declare_io(nc)
            for r in range(k):
                body(tc, io, sfx=f"_{r}" if k > 1 else "")
        nc.compile()
        _CACHED[k] = nc
    return _CACHED[k]


def build():
    return build_k(1)


def _onesc_matrix():
    o = np.zeros((P, 3), np.float32)
    o[:, 0] = 1.0
    o[0:DH, 1] = 1.0
    o[DH:P, 2] = 1.0
    return o


def _selr_matrix():
    # [P, DC*H]: selr[p, m*16+h] = 1 iff h == 2m + (p >= 64)
    s = np.zeros((P, DC, H), np.float32)
    for m in range(DC):
        s[0:DH, m, 2 * m] = 1.0
        s[DH:P, m, 2 * m + 1] = 1.0
    return np.ascontiguousarray(s.reshape(P, P))


def _selb_matrix():
    # [H, DC*P]: selb[h, m*128+p] = 1 iff h == 2m + (p >= 64)
    s = np.zeros((H, DC, P), np.float32)
    for m in range(DC):
        s[2 * m, m, 0:DH] = 1.0
        s[2 * m + 1, m, DH:P] = 1.0
    return np.ascontiguousarray(s.reshape(H, DC * P))


def _pcn(w):
    """[C*P, n] -> contiguous [P, C, n] with row c*P+p landing at [p, c]."""
    n = w.shape[1]
    return np.ascontiguousarray(w.reshape(-1, P, n).transpose(1, 0, 2))


def prep_inputs(inputs):
    """Host-side preprocessing: transpose x, split/fold weights, pack all
    inputs into two flat tensors (see _R_SEGS/_F_SEGS)."""
    f = np.float32
    x = np.asarray(inputs["x"], f)
    lcc = np.asarray(inputs["lcc_values"], f)
    w_qkv = np.asarray(inputs["w_qkv"], f)
    b_qkv = np.asarray(inputs["b_qkv"], f)
    ln1_g = np.asarray(inputs["ln1_g"], f)
    ln1_b = np.asarray(inputs["ln1_b"], f)
    ln2_g = np.asarray(inputs["ln2_g"], f)
    ln2_b = np.asarray(inputs["ln2_b"], f)
    w_ff1 = np.asarray(inputs["w_ff1"], f)
    b_ff1 = np.asarray(inputs["b_ff1"], f)

    def chunked(b):  # [D] -> [128, DC] with chunk c in column c
        return np.ascontiguousarray(b.reshape(-1, P).T)

    xt = np.ascontiguousarray(x.T)
    xt3 = _pcn(xt)
    segs_r = {
        "xt3": xt3,
        "xot3": None,  # per-core
        "wq3": _pcn(ln1_g[:, None] * w_qkv[:, 0:D]),
        "wk3": _pcn(ln1_g[:, None] * w_qkv[:, D:2 * D]),
        "wv3": _pcn(ln1_g[:, None] * w_qkv[:, 2 * D:3 * D]),
        "wo3": _pcn(np.asarray(inputs["w_out"], f)),
        "wf13": _pcn(ln2_g[:, None] * w_ff1),
        "wf23": _pcn(np.asarray(inputs["w_ff2"], f)),
        "selr3": _selr_matrix().reshape(P, DC, H),
        "selb3": _selb_matrix().reshape(H, DC, P),
        "onesc": _onesc_matrix(),
        "ones1r": np.ones((1, P), f),
        "vones": np.ones((P, KC), f),
    }
    segs_f = {
        "bq": chunked(b_qkv[0:D] + ln1_b @ w_qkv[:, 0:D]),
        "bk": chunked(b_qkv[D:2 * D] + ln1_b @ w_qkv[:, D:2 * D]),
        "bv": np.ascontiguousarray(b_qkv[2 * D:3 * D] + ln1_b @ w_qkv[:, 2 * D:3 * D]),
        "bo": chunked(np.asarray(inputs["b_out"], f)),
        "bf1": chunked(b_ff1 + ln2_b @ w_ff1),
        "bf2": chunked(np.asarray(inputs["b_ff2"], f)),
        "lcck": np.ascontiguousarray((lcc * (0.5 * LCC)).reshape(KC, P).T),
    }

    pack_r = np.empty((_R_TOT,), f)
    for name, shape in _R_SEGS:
        if name == "xot3":
            continue
        seg = segs_r[name]
        assert seg.shape == shape, (name, seg.shape, shape)
        pack_r[_R_OFFS[name]:_R_OFFS[name] + seg.size] = seg.ravel()
    pack_f = np.empty((_F_TOT,), f)
    for name, shape in _F_SEGS:
        seg = segs_f[name]
        assert seg.shape == shape, (name, seg.shape, shape)
        pack_f[_F_OFFS[name]:_F_OFFS[name] + seg.size] = seg.ravel()

    o, sz = _R_OFFS["xot3"], P * DC * LQ
    in_maps = []
    for c in range(NCORES):
        pr = pack_r.copy()
        pr[o:o + sz] = np.ascontiguousarray(
            xt3[:, :, c * LQ:(c + 1) * LQ]).ravel()
        in_maps.append({"pack_r": pr, "pack_f": pack_f})
    return in_maps


def kernel(**inputs):
    nc = build()
    in_maps = prep_inputs(inputs)
    res = run_bass_kernel_spmd(nc, in_maps, core_ids=list(range(NCORES)))
    out = np.concatenate([res.results[c]["out_t"] for c in range(NCORES)], axis=1)
    return np.ascontiguousarray(out.T).astype(np.float32)



# revision 25
# speedup vs baseline: 1.4390x; 1.3033x over previous
"""Trainium2 Bass kernel for EnhancedMultiHeadSelfAttention (dense transformer block).

Sharding: sequence-parallel over 8 cores. Each core owns L/8 = 256 query rows.
LN1 + K/V projection for all 2048 tokens are replicated on every core (cheaper
than on-chip AllReduce at this size); scores/softmax/attn@V/out-proj/LN2/FFN are
computed only for the core's own 256 rows. No collectives.

Layout: activations are kept feature-major ("transposed", [feature, token]) so
every linear layer is matmul(out=[cols, tok], lhsT=W[k,cols], rhs=actT[k,tok])
with natural weight layout and no on-device transposes. All matmuls run as
float32r (full fp32 data, bf16-rate PE throughput for free dim >= 256).

Math notes:
 - clip(scores,-10,10) never binds: |cos|*0.125 + bias in [-0.125, 0.225].
 - softmax needs no max-subtraction for the same reason.
 - the query-side half of the lcc bias is a per-query constant factor in
   exp-space and cancels in softmax normalization; only the key-side half is
   applied (as per-partition ACT bias in the exp).
 - softmax denominators come from an appended ones-column in V.
 - LN gains/biases are folded into the following matmul's weights on the host.
"""

import numpy as np

import concourse.bass as bass
import concourse.tile as tile
from concourse import bacc, mybir
from concourse.bass_utils import run_bass_kernel_spmd

F32 = mybir.dt.float32
F32R = mybir.dt.float32r
BF16 = mybir.dt.bfloat16

L = 2048          # sequence length
D = 1024          # model dim
H = 16            # heads
DH = 64           # head dim
FF = 4096         # ffn hidden
P = 128           # partitions
NCORES = 8
LQ = L // NCORES  # 256 own query rows per core
DC = D // P       # 8 d-model chunks
FC = FF // P      # 32 ffn chunks
KC = L // P       # 16 key chunks
NBLK = 4          # token blocks of 512 for the replicated phase
BLK = L // NBLK   # 512

# CoreSim doesn't implement Gelu; test_sim swaps this to Identity and checks
# against a gelu-less reference. Hardware always uses the real (erf) Gelu.
GELU_FUNC = mybir.ActivationFunctionType.Gelu

LN_EPS = 1e-5
NORM_EPS = 1e-12
SCALING = DH ** -0.5
LCC = 0.1


def _mm(nc, out, lhsT, rhs, start, stop):
    assert lhsT.dtype in (F32R, BF16) and rhs.dtype in (F32R, BF16), \
        (lhsT.dtype, rhs.dtype)
    nc.tensor.matmul(out, lhsT, rhs, start=start, stop=stop)


# ---- packed-input layout ---------------------------------------------------
# All ExternalInputs are packed into two flat DRAM tensors (pack_r: f32r
# matmul operands, pack_f: f32 bias/coef tensors). The axon client pays a
# fixed per-buffer enqueue cost (~30us) per execution, so 21 NEFF inputs
# cost ~0.7us more per exec than 2. Each segment is stored host-side
# C-contiguous in exactly the [partition, chunk, col] view shape the body
# uses, so views are simple strided APs.

# Streamed weights are stored tile-major so every DMA tile load is one
# fully-contiguous DRAM block (>= 2KB per partition line), and in bf16
# (halves the dominant DMA traffic; activations stay fp32, and the PE
# runs f32r at bf16 rate anyway so only the weight quantization error
# ~2^-9 is introduced).
QW = 256  # V quarter width
_R_SEGS = [
    ("xt_b", (NBLK, P, DC, BLK)),
    ("xot3", (P, DC, LQ)),
    ("selr3", (P, DC, H)),
    ("selb3", (H, DC, P)),
    ("onesc", (P, 3)),
    ("ones1r", (1, P)),
]
_H_SEGS = [
    ("wq_t", (DC, P, DC, P)),
    ("wk_t", (DC, P, DC, P)),
    ("wv_t", (4, P, DC, QW)),
    ("wo_t", (DC, P, DC, P)),
    ("wf1_t", (FC, P, DC, P)),
    ("wf2_t", (2, FC, P, 4, P)),
]
_F_SEGS = [
    ("bq", (P, DC)),
    ("bk", (P, DC)),
    ("bv", (D,)),
    ("bo", (P, DC)),
    ("bf1", (P, FC)),
    ("bf2", (P, DC)),
    ("lcck", (P, KC)),
]


def _seg_offsets(segs):
    offs, acc = {}, 0
    for name, shape in segs:
        offs[name] = acc
        acc += int(np.prod(shape))
    return offs, acc


_R_OFFS, _R_TOT = _seg_offsets(_R_SEGS)
_H_OFFS, _H_TOT = _seg_offsets(_H_SEGS)
_F_OFFS, _F_TOT = _seg_offsets(_F_SEGS)


def _view(pack_ap, off, shape):
    strides, acc = [], 1
    for s in reversed(shape):
        strides.append(acc)
        acc *= s
    strides = list(reversed(strides))
    return bass.AP(tensor=pack_ap.tensor, offset=off,
                   ap=[[st, sz] for st, sz in zip(strides, shape)])


def declare_io(nc):
    pack_r = nc.dram_tensor("pack_r", [_R_TOT], F32R, kind="ExternalInput").ap()
    pack_h = nc.dram_tensor("pack_h", [_H_TOT], BF16, kind="ExternalInput").ap()
    pack_f = nc.dram_tensor("pack_f", [_F_TOT], F32, kind="ExternalInput").ap()
    out_t = nc.dram_tensor("out_t", [D, LQ], F32, kind="ExternalOutput").ap()

    io = {}
    for name, shape in _R_SEGS:
        if name == "xt_b":
            n, tshape = shape[0], shape[1:]
            tsz = int(np.prod(tshape))
            io[name] = [_view(pack_r, _R_OFFS[name] + i * tsz, tshape)
                        for i in range(n)]
        else:
            io[name] = _view(pack_r, _R_OFFS[name], shape)
    for name, shape in _H_SEGS:
        if name == "wf2_t":
            ng, nf, tshape = shape[0], shape[1], shape[2:]
            tsz = int(np.prod(tshape))
            io[name] = [[_view(pack_h, _H_OFFS[name] + (g * nf + f) * tsz,
                               tshape) for f in range(nf)] for g in range(ng)]
        else:
            n, tshape = shape[0], shape[1:]
            tsz = int(np.prod(tshape))
            io[name] = [_view(pack_h, _H_OFFS[name] + i * tsz, tshape)
                        for i in range(n)]
    for name, shape in _F_SEGS:
        io[name] = _view(pack_f, _F_OFFS[name], shape)
    # b_v broadcast to all partitions: stride-0 partition dim
    io["bv_bc"] = bass.AP(tensor=pack_f.tensor, offset=_F_OFFS["bv"],
                          ap=[[0, P], [1, D]])
    io["out3"] = out_t.rearrange("(c p) t -> p c t", p=P)  # [128, 8, 256]
    io["out_t"] = out_t
    return io


def emit(tc):
    body(tc, declare_io(tc.nc))


def body(tc, io, sfx=""):
    nc = tc.nc

    xot3 = io["xot3"]        # [128, 8, 256]
    out3 = io["out3"]        # [128, 8, 256]

    # ---- persistent small constants -------------------------------------
    singles = tc.alloc_tile_pool(name="singles" + sfx, bufs=1)
    ones_1x128 = singles.tile([1, P], F32R)  # K=1 broadcast lhsT
    nc.sync.dma_start(ones_1x128, io["ones1r"])
    onesc_sb = singles.tile([P, 3], F32R)
    nc.sync.dma_start(onesc_sb, io["onesc"])
    ones_col = onesc_sb[:, 0:1]              # K=128 -> M=1 reduction lhsT
    # head-norm selectors (host-precomputed):
    # selr_sb[:, m, h] = 1 if head h belongs to chunk m at this partition;
    # selb_sb[h, m, p] = transpose, for broadcasting norms back to chunks
    selr_sb = singles.tile([P, DC, H], F32R)
    nc.sync.dma_start(selr_sb, io["selr3"])
    selb_sb = singles.tile([H, DC, P], F32R)
    nc.sync.dma_start(selb_sb, io["selb3"])
    bq_sb = singles.tile([P, DC], F32)
    nc.sync.dma_start(bq_sb, io["bq"])
    bk_sb = singles.tile([P, DC], F32)
    nc.sync.dma_start(bk_sb, io["bk"])
    bo_sb = singles.tile([P, DC], F32)
    nc.sync.dma_start(bo_sb, io["bo"])
    bf1_sb = singles.tile([P, FC], F32)
    nc.sync.dma_start(bf1_sb, io["bf1"])
    bf2_sb = singles.tile([P, DC], F32)
    nc.sync.dma_start(bf2_sb, io["bf2"])
    lcc_sb = singles.tile([P, KC], F32)
    nc.sync.dma_start(lcc_sb, io["lcck"])
    bv_sb = singles.tile([P, D], F32)  # b_v broadcast to all partitions
    nc.sync.dma_start(bv_sb, io["bv_bc"])
    eps_sb = singles.tile([1, 1], F32)
    nc.vector.memset(eps_sb, LN_EPS)

    def layer_norm_t(ctx_pool, ps_stat, ps_coef, src_tiles, dst, ncols, sq_pool,
                     src3=None, dst3=None, add_eng=None):
        """LayerNorm along feature dim for feature-major tiles.

        src_tiles: list of DC tiles/APs [128, ncols] (feature chunks)
        dst: [128, DC, ncols] output tile
        """
        sums = ps_stat.tile([1, ncols], F32, tag="stat")
        sumsq = ps_stat.tile([1, ncols], F32, tag="stat")
        for c in range(DC):
            xc = src_tiles[c]
            xsq = sq_pool.tile([P, ncols], F32R, tag="xsq")
            nc.scalar.square(xsq, xc)
            _mm(nc, sums, ones_col, xc, c == 0, c == DC - 1)
            _mm(nc, sumsq, ones_col, xsq, c == 0, c == DC - 1)
        # coeffs on one partition: rstd, shift = -mu*rstd
        mu = ctx_pool.tile([1, ncols], F32, tag="mu")
        nc.vector.tensor_scalar_mul(mu, sums, 1.0 / D)
        ex2 = ctx_pool.tile([1, ncols], F32, tag="ex2")
        nc.vector.tensor_scalar_mul(ex2, sumsq, 1.0 / D)
        var = ctx_pool.tile([1, ncols], F32, tag="var")
        nc.vector.tensor_mul(var, mu, mu)
        nc.vector.tensor_sub(var, ex2, var)
        sd = ctx_pool.tile([1, ncols], F32, tag="sd")
        nc.scalar.activation(sd, var, func=mybir.ActivationFunctionType.Sqrt,
                             bias=eps_sb, scale=1.0)
        rstd = ctx_pool.tile([1, ncols], F32R, tag="rstd")
        with nc.allow_low_precision(reason="f32r matmul operand"):
            nc.vector.reciprocal(rstd, sd)
        shift = ctx_pool.tile([1, ncols], F32R, tag="shift")
        nc.vector.tensor_mul(shift, mu, rstd)
        nc.vector.tensor_scalar_mul(shift, shift, -1.0)
        # broadcast to 128 partitions via K=1 matmul
        rstd_bc = ps_coef.tile([P, ncols], F32, tag="coef")
        shift_bc = ps_coef.tile([P, ncols], F32, tag="coef")
        _mm(nc, rstd_bc, ones_1x128, rstd, True, True)
        _mm(nc, shift_bc, ones_1x128, shift, True, True)
        if dst3 is not None:
            # one 3D op per pass; alternate the add between DVE and GpSimd so
            # neither engine serializes the block pipeline. GpSimd cannot read
            # PSUM, so stage the shift coefficients through SBUF for it.
            rb = rstd_bc.unsqueeze(1).to_broadcast(dst3.shape)
            if add_eng is nc.gpsimd:
                shift_sb = ctx_pool.tile([P, ncols], F32, tag="shift_sb",
                                         bufs=2)
                nc.scalar.copy(shift_sb, shift_bc)
                sb = shift_sb.unsqueeze(1).to_broadcast(dst3.shape)
            else:
                sb = shift_bc.unsqueeze(1).to_broadcast(dst3.shape)
            nc.vector.tensor_mul(dst3, src3, rb)
            add_eng.tensor_add(dst3, dst3, sb)
        else:
            for c in range(DC):
                nc.vector.tensor_mul(dst[:, c, :], src_tiles[c], rstd_bc)
                nc.vector.tensor_add(dst[:, c, :], dst[:, c, :], shift_bc)


    # persistent pools, allocated in reverse-release (stack) order
    vdram_pool = tc.alloc_tile_pool(name="vdram", bufs=1, space="DRAM")
    # V scratch, head-pair-major: [pair m][token-in-chunk p][chunk][j][d]
    # so phase D reads one pair as a single contiguous [P, KC*2*DH] DMA.
    v_dram = vdram_pool.tile([DC, P, KC, 2, DH], F32R)
    x2_pool = tc.alloc_tile_pool(name="x2p", bufs=1)
    x2acc = x2_pool.tile([P, DC, LQ], F32)
    x2 = x2_pool.tile([P, DC, LQ], F32R)
    kt_pool = tc.alloc_tile_pool(name="kt", bufs=1)
    k_t = kt_pool.tile([P, DC, L], F32R)  # [col-in-chunk, chunk, token]
    q_pool = tc.alloc_tile_pool(name="q", bufs=1)
    q_t = q_pool.tile([P, DC, LQ], F32R)
    normed_pool = tc.alloc_tile_pool(name="normed", bufs=1)
    normed_full = normed_pool.tile([P, DC, L], F32R)

    # =====================================================================
    # Phase A: LN1 over all tokens -> normed_full (feature-major, in place)
    # =====================================================================
    with (
        tc.tile_pool(name="ln1sq", bufs=2) as sq_pool,
        tc.tile_pool(name="ln1coef", bufs=1) as coef_small,
        tc.tile_pool(name="ps_stat", bufs=4, space="PSUM") as ps_stat,
        tc.tile_pool(name="ps_coef", bufs=2, space="PSUM") as ps_coef,
    ):
        for b in range(NBLK):
            blk = normed_full[:, :, b * BLK:(b + 1) * BLK]
            eng = nc.sync if b % 2 == 0 else nc.gpsimd
            eng.dma_start(blk, io["xt_b"][b])
            layer_norm_t(coef_small, ps_stat, ps_coef,
                         [blk[:, c, :] for c in range(DC)], blk, BLK, sq_pool,
                         src3=blk, dst3=blk,
                         add_eng=nc.gpsimd if b % 2 == 0 else nc.vector)

    # =====================================================================
    # Phase C: own queries: LN1(own) -> q^T -> cosine-normalize * scaling
    # =====================================================================
    with (
        tc.tile_pool(name="qb", bufs=1) as qb_pool,
        tc.tile_pool(name="qsq", bufs=2) as qsq_pool,
        tc.tile_pool(name="qcoef", bufs=1) as qcoef,
        tc.tile_pool(name="wqstream", bufs=2) as wqstream,
    ):
        normed_own = qb_pool.tile([P, DC, LQ], F32R)
        nc.sync.dma_start(normed_own, xot3)
        with (
            tc.tile_pool(name="ps_stat2", bufs=2, space="PSUM") as ps_stat2,
            tc.tile_pool(name="ps_coef2", bufs=2, space="PSUM") as ps_coef2,
        ):
            layer_norm_t(qcoef, ps_stat2, ps_coef2,
                         [normed_own[:, c, :] for c in range(DC)], normed_own, LQ,
                         qsq_pool)
        with (
            tc.tile_pool(name="ps_mm2", bufs=2, space="PSUM") as ps_mm2,
            tc.tile_pool(name="ps_qn", bufs=2, space="PSUM") as ps_qn,
            tc.tile_pool(name="ps_qbc", bufs=2, space="PSUM") as ps_qbc,
        ):
            for m in range(DC):
                wqm = wqstream.tile([P, DC, P], F32R, tag="wq")
                nc.sync.dma_start(wqm, io["wq_t"][m])
                ps = ps_mm2.tile([P, LQ], F32, tag="mm")
                for c in range(DC):
                    _mm(nc, ps, wqm[:, c, :], normed_own[:, c, :], c == 0,
                        c == DC - 1)
                nc.vector.tensor_scalar_add(q_t[:, m, :], ps, bq_sb[:, m:m + 1])
            # cosine-normalize q (x scaling folded into reciprocal)
            nsq = ps_qn.tile([H, LQ], F32, tag="qnsq")
            for m in range(DC):
                qsq = qsq_pool.tile([P, LQ], F32R, tag="xsq")
                nc.scalar.square(qsq, q_t[:, m, :])
                _mm(nc, nsq, selr_sb[:, m, :], qsq, m == 0, m == DC - 1)
            sd = qcoef.tile([H, LQ], F32, tag="qsd", bufs=2)
            nc.scalar.activation(sd, nsq,
                                 func=mybir.ActivationFunctionType.Sqrt,
                                 bias=0.0, scale=1.0)
            nc.vector.tensor_scalar_max(sd, sd, NORM_EPS)
            rec = qcoef.tile([H, LQ], F32R, tag="qrec", bufs=2)
            with nc.allow_low_precision(reason="f32r matmul operand"):
                nc.vector.reciprocal(rec, sd)
            nc.vector.tensor_scalar_mul(rec, rec, SCALING)
            for m in range(DC):
                bc = ps_qbc.tile([P, LQ], F32, tag="qbc")
                _mm(nc, bc, selb_sb[:, m, :], rec, True, True)
                nc.vector.tensor_mul(q_t[:, m, :], q_t[:, m, :], bc)

    # =====================================================================
    # Phase B: V (to DRAM scratch) then K^T + cosine-norm, block-pipelined
    # =====================================================================
    with (
        tc.tile_pool(name="wstream", bufs=2) as wstream,
        tc.tile_pool(name="vstage", bufs=3) as vstage,
        tc.tile_pool(name="knorm", bufs=2) as knorm_pool,
        tc.tile_pool(name="ps_mm", bufs=4, space="PSUM") as ps_mm,
        tc.tile_pool(name="ps_nrm", bufs=1, space="PSUM") as ps_nrm,
        tc.tile_pool(name="ps_nbc", bufs=1, space="PSUM") as ps_nbc,
    ):
        # V natural layout, block-major inside each quarter so the first
        # blocks of normed unblock V matmuls early
        for n in range(4):
            wvn = wstream.tile([P, DC, QW], F32R, tag="wv")
            nc.gpsimd.dma_start(wvn, io["wv_t"][n])
            for t in range(KC):
                ps = ps_mm.tile([P, QW], F32, tag="mmv", bufs=2)
                for c in range(DC):
                    _mm(nc, ps, normed_full[:, c, t * P:(t + 1) * P],
                        wvn[:, c, :], c == 0, c == DC - 1)
                stag = vstage.tile([P, 4, DH], F32R, tag="vstage")
                nc.vector.tensor_add(
                    stag, ps.rearrange("p (h d) -> p h d", d=DH),
                    bv_sb[:, n * QW:(n + 1) * QW].rearrange("p (h d) -> p h d",
                                                            d=DH))
                for hp in range(2):
                    nc.gpsimd.dma_start(
                        v_dram[2 * n + hp, :, t, :, :],
                        stag[:, 2 * hp:2 * hp + 2, :])
        # K projection: m-outer so each wk tile is fetched exactly once
        for m in range(DC):
            wkm = wstream.tile([P, DC, P], F32R, tag="wk")
            nc.sync.dma_start(wkm, io["wk_t"][m])
            for b in range(NBLK):
                ps = ps_mm.tile([P, BLK], F32, tag="mm")
                for c in range(DC):
                    _mm(nc, ps, wkm[:, c, :],
                        normed_full[:, c, b * BLK:(b + 1) * BLK], c == 0,
                        c == DC - 1)
                nc.vector.tensor_scalar_add(k_t[:, m, b * BLK:(b + 1) * BLK],
                                            ps, bk_sb[:, m:m + 1])
        # cosine-normalize K blockwise
        for b in range(NBLK):
            nsq = ps_nrm.tile([H, BLK], F32, tag="nsq")
            for m in range(DC):
                ksq = knorm_pool.tile([P, BLK], F32R, tag="ksq")
                nc.scalar.square(ksq, k_t[:, m, b * BLK:(b + 1) * BLK])
                _mm(nc, nsq, selr_sb[:, m, :], ksq, m == 0, m == DC - 1)
            sd = knorm_pool.tile([H, BLK], F32, tag="ksd")
            nc.scalar.activation(sd, nsq,
                                 func=mybir.ActivationFunctionType.Sqrt,
                                 bias=0.0, scale=1.0)
            nc.vector.tensor_scalar_max(sd, sd, NORM_EPS)
            rec = knorm_pool.tile([H, BLK], F32R, tag="krec")
            with nc.allow_low_precision(reason="f32r matmul operand"):
                nc.vector.reciprocal(rec, sd)
            for m in range(DC):
                bc = ps_nbc.tile([P, BLK], F32, tag="nbc")
                _mm(nc, bc, selb_sb[:, m, :], rec, True, True)
                nc.vector.tensor_mul(k_t[:, m, b * BLK:(b + 1) * BLK],
                                     k_t[:, m, b * BLK:(b + 1) * BLK], bc)

    normed_pool.release()

    # =====================================================================
    # Phase D: attention per head-pair, with the out-projection folded in
    # (partial products accumulated into x2acc via DVE)
    # =====================================================================
    with (
        tc.tile_pool(name="exp", bufs=2) as exp_pool,
        tc.tile_pool(name="vsb", bufs=2) as vsb_pool,
        tc.tile_pool(name="rsc", bufs=2) as rsc_pool,
        tc.tile_pool(name="apair", bufs=2) as apair_pool,
        tc.tile_pool(name="wostream", bufs=2) as wostream,
        tc.tile_pool(name="ps_sc", bufs=2, space="PSUM") as ps_sc,
        tc.tile_pool(name="ps_acc", bufs=1, space="PSUM") as ps_acc,
        tc.tile_pool(name="ps_rbc", bufs=1, space="PSUM") as ps_rbc,
        tc.tile_pool(name="ps_op", bufs=2, space="PSUM") as ps_op,
    ):
        for m in range(DC):
            vp = vsb_pool.tile([P, KC, 2, DH + 1], F32R, tag="vp")
            nc.gpsimd.dma_start(vp[:, :, :, 0:DH], v_dram[m])
            nc.vector.tensor_copy(
                vp[:, :, :, DH:DH + 1],
                ones_col.unsqueeze(1).unsqueeze(1).to_broadcast((P, KC, 2, 1)))
            eh = exp_pool.tile([P, KC, 2 * LQ], F32R, tag="exp")
            for kc in range(KC):
                # each head's scores go to a separate PSUM bank: fp32r matmul
                # writes at mid-bank free offsets fault on hardware
                ps = ps_sc.tile([P, 2, 2 * LQ], F32, tag="sc")
                for j in range(2):
                    _mm(nc, ps[:, j, 0:LQ],
                        k_t[j * DH:(j + 1) * DH, m, kc * P:(kc + 1) * P],
                        q_t[j * DH:(j + 1) * DH, m, :], True, True)
                nc.scalar.activation(
                    eh[:, kc, :].rearrange("p (j q) -> p j q", j=2),
                    ps[:, :, 0:LQ],
                    func=mybir.ActivationFunctionType.Exp,
                    bias=lcc_sb[:, kc:kc + 1], scale=1.0)
            attn_pair = apair_pool.tile([P, LQ], F32R, tag="apair")
            for j in range(2):
                acc = ps_acc.tile([DH + 1, LQ], F32, tag="acc")
                for kc in range(KC):
                    _mm(nc, acc, vp[:, kc, j, :],
                        eh[:, kc, j * LQ:(j + 1) * LQ], kc == 0, kc == KC - 1)
                recip = rsc_pool.tile([1, LQ], F32R, tag="recip")
                with nc.allow_low_precision(reason="f32r matmul operand"):
                    nc.vector.reciprocal(recip, acc[DH:DH + 1, :])
                rbc = ps_rbc.tile([DH, LQ], F32, tag="rbc")
                _mm(nc, rbc, ones_1x128[:, 0:DH], recip, True, True)
                rbc_sb = rsc_pool.tile([DH, LQ], F32, tag="rbcsb")
                nc.vector.tensor_copy(rbc_sb, rbc)
                nc.vector.tensor_mul(attn_pair[j * DH:(j + 1) * DH, :],
                                     acc[0:DH, :], rbc_sb)
            # out-projection partial for this pair-chunk of attn
            wom = wostream.tile([P, DC, P], F32R, tag="wo")
            nc.sync.dma_start(wom, io["wo_t"][m])
            for o in range(DC):
                pso = ps_op.tile([P, LQ], F32, tag="op")
                _mm(nc, pso, wom[:, o, :], attn_pair, True, True)
                if m == 0:
                    nc.vector.tensor_copy(x2acc[:, o, :], pso)
                else:
                    nc.vector.tensor_add(x2acc[:, o, :], x2acc[:, o, :], pso)

    q_pool.release()
    kt_pool.release()

    # =====================================================================
    # Phase E: residual -> x2; LN2; FFN (ff2 single-pass, half-packed psum)
    # =====================================================================
    with (
        tc.tile_pool(name="xo2p", bufs=1) as xo2_pool,
        tc.tile_pool(name="ffsq", bufs=2) as ffsq_pool,
        tc.tile_pool(name="ffcoef", bufs=2) as ffcoef,
        tc.tile_pool(name="ht", bufs=1) as ht_pool,
        tc.tile_pool(name="wf1s", bufs=3) as wf1s,
        tc.tile_pool(name="wf2s", bufs=3) as wf2s,
        tc.tile_pool(name="outsb", bufs=2) as outsb_pool,
    ):
        xo2 = xo2_pool.tile([P, DC, LQ], F32R)
        nc.sync.dma_start(xo2, xot3)
        for o in range(DC):
            nc.vector.tensor_scalar_add(x2[:, o, :], x2acc[:, o, :],
                                        bo_sb[:, o:o + 1])
            nc.vector.tensor_add(x2[:, o, :], x2[:, o, :], xo2[:, o, :])
        normed2 = xo2_pool.tile([P, DC, LQ], F32R)
        with (
            tc.tile_pool(name="ps_stat3", bufs=2, space="PSUM") as ps_stat3,
            tc.tile_pool(name="ps_coef3", bufs=2, space="PSUM") as ps_coef3,
        ):
            layer_norm_t(ffcoef, ps_stat3, ps_coef3,
                         [x2[:, c, :] for c in range(DC)], normed2, LQ,
                         ffsq_pool)
        ps_mm3 = tc.alloc_tile_pool(name="ps_mm3", bufs=3, space="PSUM")
        ps_ff2 = tc.alloc_tile_pool(name="ps_ff2", bufs=4, space="PSUM")
        h_t = ht_pool.tile([P, FC, LQ], F32R)
        for f in range(FC):
            wf1m = wf1s.tile([P, DC, P], F32R, tag="wf1")
            weng = nc.sync if f % 2 == 0 else nc.gpsimd
            weng.dma_start(wf1m, io["wf1_t"][f])
            ps = ps_mm3.tile([P, LQ], F32, tag="mm")
            for c in range(DC):
                _mm(nc, ps, wf1m[:, c, :], normed2[:, c, :], c == 0, c == DC - 1)
            nc.scalar.activation(h_t[:, f, :], ps, func=GELU_FUNC,
                                 bias=bf1_sb[:, f:f + 1], scale=1.0)
        # ff2: f-outer accumulation in two 4-output passes; pass 1 pipelines
        # with ff1 chunk by chunk
        for g in range(2):
            accs = [ps_ff2.tile([P, LQ], F32, tag="ff2acc",
                                name=f"ff2acc_{g}_{i}") for i in range(4)]
            for f in range(FC):
                wf2m = wf2s.tile([P, 4, P], F32R, tag="wf2")
                weng2 = nc.gpsimd if f % 2 == 0 else nc.sync
                weng2.dma_start(wf2m, io["wf2_t"][g][f])
                for i in range(4):
                    _mm(nc, accs[i], wf2m[:, i, :], h_t[:, f, :],
                        f == 0, f == FC - 1)
            for i in range(4):
                mcol = g * 4 + i
                osb = outsb_pool.tile([P, LQ], F32, tag="osb")
                nc.vector.tensor_scalar_add(osb, accs[i], bf2_sb[:, mcol:mcol + 1])
                nc.vector.tensor_add(osb, osb, x2[:, mcol, :])
                nc.sync.dma_start(out3[:, mcol, :], osb)
        ps_ff2.release()
        ps_mm3.release()

    x2_pool.release()
    vdram_pool.release()
    singles.release()


_CACHED = {}


def build_k(k=1):
    """Build a NEFF with the kernel body emitted k times back-to-back.

    k=1 is the production kernel; k>1 exists so a timing harness can
    measure steady-state per-execution time on-device (the repetitions
    are data-independent re-runs writing the same output)."""
    if k not in _CACHED:
        nc = bacc.Bacc("TRN2", target_bir_lowering=False, debug=False)
        with tile.TileContext(nc) as tc:
            io = declare_io(nc)
            for r in range(k):
                body(tc, io, sfx=f"_{r}" if k > 1 else "")
        nc.compile()
        _CACHED[k] = nc
    return _CACHED[k]


def build():
    return build_k(1)


def _onesc_matrix():
    o = np.zeros((P, 3), np.float32)
    o[:, 0] = 1.0
    o[0:DH, 1] = 1.0
    o[DH:P, 2] = 1.0
    return o


def _selr_matrix():
    # [P, DC*H]: selr[p, m*16+h] = 1 iff h == 2m + (p >= 64)
    s = np.zeros((P, DC, H), np.float32)
    for m in range(DC):
        s[0:DH, m, 2 * m] = 1.0
        s[DH:P, m, 2 * m + 1] = 1.0
    return np.ascontiguousarray(s.reshape(P, P))


def _selb_matrix():
    # [H, DC*P]: selb[h, m*128+p] = 1 iff h == 2m + (p >= 64)
    s = np.zeros((H, DC, P), np.float32)
    for m in range(DC):
        s[2 * m, m, 0:DH] = 1.0
        s[2 * m + 1, m, DH:P] = 1.0
    return np.ascontiguousarray(s.reshape(H, DC * P))


def _pcn(w):
    """[C*P, n] -> contiguous [P, C, n] with row c*P+p landing at [p, c]."""
    n = w.shape[1]
    return np.ascontiguousarray(w.reshape(-1, P, n).transpose(1, 0, 2))


def prep_inputs(inputs):
    """Host-side preprocessing: transpose x, split/fold weights, pack all
    inputs into two flat tensors (see _R_SEGS/_F_SEGS)."""
    f = np.float32
    x = np.asarray(inputs["x"], f)
    lcc = np.asarray(inputs["lcc_values"], f)
    w_qkv = np.asarray(inputs["w_qkv"], f)
    b_qkv = np.asarray(inputs["b_qkv"], f)
    ln1_g = np.asarray(inputs["ln1_g"], f)
    ln1_b = np.asarray(inputs["ln1_b"], f)
    ln2_g = np.asarray(inputs["ln2_g"], f)
    ln2_b = np.asarray(inputs["ln2_b"], f)
    w_ff1 = np.asarray(inputs["w_ff1"], f)
    b_ff1 = np.asarray(inputs["b_ff1"], f)

    def chunked(b):  # [D] -> [128, DC] with chunk c in column c
        return np.ascontiguousarray(b.reshape(-1, P).T)

    def tiled(w, ncols):  # [D, n] -> [n/ncols, P, DC, ncols] tile-major
        return np.ascontiguousarray(
            w.reshape(DC, P, w.shape[1] // ncols, ncols).transpose(2, 1, 0, 3))

    xt = np.ascontiguousarray(x.T)
    xt3 = _pcn(xt)
    wf2 = np.asarray(inputs["w_ff2"], f)
    segs_r = {
        "xt_b": np.ascontiguousarray(
            xt3.reshape(P, DC, NBLK, BLK).transpose(2, 0, 1, 3)),
        "xot3": None,  # per-core
        "wq_t": tiled(ln1_g[:, None] * w_qkv[:, 0:D], P),
        "wk_t": tiled(ln1_g[:, None] * w_qkv[:, D:2 * D], P),
        "wv_t": tiled(ln1_g[:, None] * w_qkv[:, 2 * D:3 * D], QW),
        "wo_t": np.asarray(inputs["w_out"], f).reshape(DC, P, DC, P),
        "wf1_t": tiled(ln2_g[:, None] * w_ff1, P),
        "wf2_t": np.ascontiguousarray(
            wf2.reshape(FC, P, 2, 4, P).transpose(2, 0, 1, 3, 4)),
        "selr3": _selr_matrix().reshape(P, DC, H),
        "selb3": _selb_matrix().reshape(H, DC, P),
        "onesc": _onesc_matrix(),
        "ones1r": np.ones((1, P), f),
    }
    segs_f = {
        "bq": chunked(b_qkv[0:D] + ln1_b @ w_qkv[:, 0:D]),
        "bk": chunked(b_qkv[D:2 * D] + ln1_b @ w_qkv[:, D:2 * D]),
        "bv": np.ascontiguousarray(b_qkv[2 * D:3 * D] + ln1_b @ w_qkv[:, 2 * D:3 * D]),
        "bo": chunked(np.asarray(inputs["b_out"], f)),
        "bf1": chunked(b_ff1 + ln2_b @ w_ff1),
        "bf2": chunked(np.asarray(inputs["b_ff2"], f)),
        "lcck": np.ascontiguousarray((lcc * (0.5 * LCC)).reshape(KC, P).T),
    }

    pack_r = np.empty((_R_TOT,), f)
    for name, shape in _R_SEGS:
        if name == "xot3":
            continue
        seg = segs_r[name]
        assert seg.shape == shape, (name, seg.shape, shape)
        pack_r[_R_OFFS[name]:_R_OFFS[name] + seg.size] = seg.ravel()
    pack_f = np.empty((_F_TOT,), f)
    for name, shape in _F_SEGS:
        seg = segs_f[name]
        assert seg.shape == shape, (name, seg.shape, shape)
        pack_f[_F_OFFS[name]:_F_OFFS[name] + seg.size] = seg.ravel()

    o, sz = _R_OFFS["xot3"], P * DC * LQ
    in_maps = []
    for c in range(NCORES):
        pr = pack_r.copy()
        pr[o:o + sz] = np.ascontiguousarray(
            xt3[:, :, c * LQ:(c + 1) * LQ]).ravel()
        in_maps.append({"pack_r": pr, "pack_f": pack_f})
    return in_maps


def kernel(**inputs):
    nc = build()
    in_maps = prep_inputs(inputs)
    res = run_bass_kernel_spmd(nc, in_maps, core_ids=list(range(NCORES)))
    out = np.concatenate([res.results[c]["out_t"] for c in range(NCORES)], axis=1)
    return np.ascontiguousarray(out.T).astype(np.float32)



# revision 34
# speedup vs baseline: 2.5049x; 1.7406x over previous
"""Trainium2 Bass kernel for EnhancedMultiHeadSelfAttention (dense transformer block).

Sharding: sequence-parallel over 8 cores. Each core owns L/8 = 256 query rows.
LN1 + K/V projection for all 2048 tokens are replicated on every core (cheaper
than on-chip AllReduce at this size); scores/softmax/attn@V/out-proj/LN2/FFN are
computed only for the core's own 256 rows. No collectives.

Layout: activations are kept feature-major ("transposed", [feature, token]) so
every linear layer is matmul(out=[cols, tok], lhsT=W[k,cols], rhs=actT[k,tok])
with natural weight layout and no on-device transposes. All matmuls run as
float32r (full fp32 data, bf16-rate PE throughput for free dim >= 256).

Math notes:
 - clip(scores,-10,10) never binds: |cos|*0.125 + bias in [-0.125, 0.225].
 - softmax needs no max-subtraction for the same reason.
 - the query-side half of the lcc bias is a per-query constant factor in
   exp-space and cancels in softmax normalization; only the key-side half is
   applied (as per-partition ACT bias in the exp).
 - softmax denominators come from an appended ones-column in V.
 - LN gains/biases are folded into the following matmul's weights on the host.
"""

import numpy as np

import concourse.bass as bass
import concourse.tile as tile
from concourse import bacc, mybir
from concourse.bass_utils import run_bass_kernel_spmd

F32 = mybir.dt.float32
F32R = mybir.dt.float32r
BF16 = mybir.dt.bfloat16

L = 2048          # sequence length
D = 1024          # model dim
H = 16            # heads
DH = 64           # head dim
FF = 4096         # ffn hidden
P = 128           # partitions
NCORES = 8
LQ = L // NCORES  # 256 own query rows per core
DC = D // P       # 8 d-model chunks
FC = FF // P      # 32 ffn chunks
KC = L // P       # 16 key chunks
NBLK = 4          # token blocks of 512 for the replicated phase
BLK = L // NBLK   # 512

# CoreSim doesn't implement Gelu; test_sim swaps this to Identity and checks
# against a gelu-less reference. Hardware always uses the real (erf) Gelu.
GELU_FUNC = mybir.ActivationFunctionType.Gelu

LN_EPS = 1e-5
NORM_EPS = 1e-12
SCALING = DH ** -0.5
LCC = 0.1


def _mm(nc, out, lhsT, rhs, start, stop):
    # HW matmul forbids mixing f32/f32r with 16-bit operands
    assert lhsT.dtype in (F32R, BF16) and rhs.dtype == lhsT.dtype, \
        (lhsT.dtype, rhs.dtype)
    nc.tensor.matmul(out, lhsT, rhs, start=start, stop=stop)


# ---- packed-input layout ---------------------------------------------------
# All ExternalInputs are packed into two flat DRAM tensors (pack_r: f32r
# matmul operands, pack_f: f32 bias/coef tensors). The axon client pays a
# fixed per-buffer enqueue cost (~30us) per execution, so 21 NEFF inputs
# cost ~0.7us more per exec than 2. Each segment is stored host-side
# C-contiguous in exactly the [partition, chunk, col] view shape the body
# uses, so views are simple strided APs.

# Streamed weights are stored tile-major so every DMA tile load is one
# fully-contiguous DRAM block (>= 2KB per partition line), and in bf16
# (halves the dominant DMA traffic; activations stay fp32, and the PE
# runs f32r at bf16 rate anyway so only the weight quantization error
# ~2^-9 is introduced).
QW = 256  # V quarter width
_R_SEGS = [
    ("xot3", (P, DC, LQ)),
    ("wq_t", (DC, P, DC, P)),   # f32r: keeps the Q path full-precision
    ("selr3", (P, DC, H)),
    ("selb3", (H, DC, P)),
    ("onesc", (P, 3)),
    ("ones1r", (1, P)),
]
_H_SEGS = [
    ("xt_b", (NBLK, P, DC, BLK)),
    ("wk_t", (DC, P, DC, P)),
    ("wv_t", (4, P, DC, QW)),
    ("wo_t", (DC, P, DC, P)),
    ("wf1_t", (FC, P, DC, P)),
    ("wf2_t", (2, FC, P, 4, P)),
    ("onesc_h", (P, 3)),
]
_F_SEGS = [
    ("bq", (P, DC)),
    ("bk", (P, DC)),
    ("bv", (D,)),
    ("bo", (P, DC)),
    ("bf1", (P, FC)),
    ("bf2", (P, DC)),
    ("lcck", (P, KC)),
]


def _seg_offsets(segs):
    offs, acc = {}, 0
    for name, shape in segs:
        offs[name] = acc
        acc += int(np.prod(shape))
    return offs, acc


_R_OFFS, _R_TOT = _seg_offsets(_R_SEGS)
_H_OFFS, _H_TOT = _seg_offsets(_H_SEGS)
_F_OFFS, _F_TOT = _seg_offsets(_F_SEGS)


def _view(pack_ap, off, shape):
    strides, acc = [], 1
    for s in reversed(shape):
        strides.append(acc)
        acc *= s
    strides = list(reversed(strides))
    return bass.AP(tensor=pack_ap.tensor, offset=off,
                   ap=[[st, sz] for st, sz in zip(strides, shape)])


def declare_io(nc):
    pack_r = nc.dram_tensor("pack_r", [_R_TOT], F32R, kind="ExternalInput").ap()
    pack_h = nc.dram_tensor("pack_h", [_H_TOT], BF16, kind="ExternalInput").ap()
    pack_f = nc.dram_tensor("pack_f", [_F_TOT], F32, kind="ExternalInput").ap()
    out_t = nc.dram_tensor("out_t", [D, LQ], F32, kind="ExternalOutput").ap()

    def seg_views(pack, offs, name, shape):
        if name == "wf2_t":
            ng, nf, tshape = shape[0], shape[1], shape[2:]
            tsz = int(np.prod(tshape))
            return [[_view(pack, offs[name] + (g * nf + f) * tsz, tshape)
                     for f in range(nf)] for g in range(ng)]
        if name in ("xt_b", "wq_t", "wk_t", "wv_t", "wo_t", "wf1_t"):
            n, tshape = shape[0], shape[1:]
            tsz = int(np.prod(tshape))
            return [_view(pack, offs[name] + i * tsz, tshape)
                    for i in range(n)]
        return _view(pack, offs[name], shape)

    io = {}
    for name, shape in _R_SEGS:
        io[name] = seg_views(pack_r, _R_OFFS, name, shape)
    for name, shape in _H_SEGS:
        io[name] = seg_views(pack_h, _H_OFFS, name, shape)
    for name, shape in _F_SEGS:
        io[name] = _view(pack_f, _F_OFFS[name], shape)
    # b_v broadcast to all partitions: stride-0 partition dim
    io["bv_bc"] = bass.AP(tensor=pack_f.tensor, offset=_F_OFFS["bv"],
                          ap=[[0, P], [1, D]])
    io["out3"] = out_t.rearrange("(c p) t -> p c t", p=P)  # [128, 8, 256]
    io["out_t"] = out_t
    return io


def emit(tc):
    body(tc, declare_io(tc.nc))


def body(tc, io, sfx=""):
    nc = tc.nc

    xot3 = io["xot3"]        # [128, 8, 256]
    out3 = io["out3"]        # [128, 8, 256]

    # ---- persistent small constants -------------------------------------
    singles = tc.alloc_tile_pool(name="singles" + sfx, bufs=1)
    ones_1x128 = singles.tile([1, P], F32R)  # K=1 broadcast lhsT
    nc.sync.dma_start(ones_1x128, io["ones1r"])
    onesc_sb = singles.tile([P, 3], F32R)
    nc.sync.dma_start(onesc_sb, io["onesc"])
    ones_col = onesc_sb[:, 0:1]              # K=128 -> M=1 reduction lhsT
    onesc_hb = singles.tile([P, 3], BF16)
    nc.sync.dma_start(onesc_hb, io["onesc_h"])
    ones_col_bf = onesc_hb[:, 0:1]
    # head-norm selectors (host-precomputed):
    # selr_sb[:, m, h] = 1 if head h belongs to chunk m at this partition;
    # selb_sb[h, m, p] = transpose, for broadcasting norms back to chunks
    selr_sb = singles.tile([P, DC, H], F32R)
    nc.sync.dma_start(selr_sb, io["selr3"])
    selb_sb = singles.tile([H, DC, P], F32R)
    nc.sync.dma_start(selb_sb, io["selb3"])
    bq_sb = singles.tile([P, DC], F32)
    nc.sync.dma_start(bq_sb, io["bq"])
    bk_sb = singles.tile([P, DC], F32)
    nc.sync.dma_start(bk_sb, io["bk"])
    bo_sb = singles.tile([P, DC], F32)
    nc.sync.dma_start(bo_sb, io["bo"])
    bf1_sb = singles.tile([P, FC], F32)
    nc.sync.dma_start(bf1_sb, io["bf1"])
    bf2_sb = singles.tile([P, DC], F32)
    nc.sync.dma_start(bf2_sb, io["bf2"])
    lcc_sb = singles.tile([P, KC], F32)
    nc.sync.dma_start(lcc_sb, io["lcck"])
    bv_sb = singles.tile([P, D], F32)  # b_v broadcast to all partitions
    nc.sync.dma_start(bv_sb, io["bv_bc"])
    eps_sb = singles.tile([1, 1], F32)
    nc.vector.memset(eps_sb, LN_EPS)

    def layer_norm_t(ctx_pool, ps_stat, ps_coef, src_tiles, dst, ncols, sq_pool,
                     src3=None, dst3=None, add_eng=None, ones_lhs=None):
        """LayerNorm along feature dim for feature-major tiles.

        src_tiles: list of DC tiles/APs [128, ncols] (feature chunks)
        dst: [128, DC, ncols] output tile
        """
        if ones_lhs is None:
            ones_lhs = ones_col
        sums = ps_stat.tile([1, ncols], F32, tag="stat")
        sumsq = ps_stat.tile([1, ncols], F32, tag="stat")
        for c in range(DC):
            xc = src_tiles[c]
            xsq = sq_pool.tile([P, ncols], xc.dtype, tag="xsq")
            nc.scalar.square(xsq, xc)
            _mm(nc, sums, ones_lhs, xc, c == 0, c == DC - 1)
            _mm(nc, sumsq, ones_lhs, xsq, c == 0, c == DC - 1)
        # coeffs on one partition: rstd, shift = -mu*rstd
        mu = ctx_pool.tile([1, ncols], F32, tag="mu")
        nc.vector.tensor_scalar_mul(mu, sums, 1.0 / D)
        ex2 = ctx_pool.tile([1, ncols], F32, tag="ex2")
        nc.vector.tensor_scalar_mul(ex2, sumsq, 1.0 / D)
        var = ctx_pool.tile([1, ncols], F32, tag="var")
        nc.vector.tensor_mul(var, mu, mu)
        nc.vector.tensor_sub(var, ex2, var)
        sd = ctx_pool.tile([1, ncols], F32, tag="sd")
        nc.scalar.activation(sd, var, func=mybir.ActivationFunctionType.Sqrt,
                             bias=eps_sb, scale=1.0)
        rstd = ctx_pool.tile([1, ncols], F32R, tag="rstd")
        with nc.allow_low_precision(reason="f32r matmul operand"):
            nc.vector.reciprocal(rstd, sd)
        shift = ctx_pool.tile([1, ncols], F32R, tag="shift")
        nc.vector.tensor_mul(shift, mu, rstd)
        nc.vector.tensor_scalar_mul(shift, shift, -1.0)
        # broadcast to 128 partitions via K=1 matmul
        rstd_bc = ps_coef.tile([P, ncols], F32, tag="coef")
        shift_bc = ps_coef.tile([P, ncols], F32, tag="coef")
        _mm(nc, rstd_bc, ones_1x128, rstd, True, True)
        _mm(nc, shift_bc, ones_1x128, shift, True, True)
        if dst3 is not None:
            # one 3D op per pass; alternate the add between DVE and GpSimd so
            # neither engine serializes the block pipeline. GpSimd cannot read
            # PSUM, so stage the shift coefficients through SBUF for it.
            rb = rstd_bc.unsqueeze(1).to_broadcast(dst3.shape)
            if add_eng is nc.gpsimd:
                shift_sb = ctx_pool.tile([P, ncols], F32, tag="shift_sb",
                                         bufs=2)
                nc.scalar.copy(shift_sb, shift_bc)
                sb = shift_sb.unsqueeze(1).to_broadcast(dst3.shape)
            else:
                sb = shift_bc.unsqueeze(1).to_broadcast(dst3.shape)
            nc.vector.tensor_mul(dst3, src3, rb)
            add_eng.tensor_add(dst3, dst3, sb)
        else:
            for c in range(DC):
                nc.vector.tensor_mul(dst[:, c, :], src_tiles[c], rstd_bc)
                nc.vector.tensor_add(dst[:, c, :], dst[:, c, :], shift_bc)


    # persistent pools, allocated in reverse-release (stack) order
    vdram_pool = tc.alloc_tile_pool(name="vdram", bufs=1, space="DRAM")
    # V scratch, head-pair-major: [pair m][token-in-chunk p][chunk][j][d]
    # so phase D reads one pair as a single contiguous [P, KC*2*DH] DMA.
    v_dram = vdram_pool.tile([DC, P, KC, 2, DH], BF16)
    x2_pool = tc.alloc_tile_pool(name="x2p", bufs=1)
    x2acc = x2_pool.tile([P, DC, LQ], F32)
    x2 = x2_pool.tile([P, DC, LQ], F32R)
    kt_pool = tc.alloc_tile_pool(name="kt", bufs=1)
    k_t = kt_pool.tile([P, DC, L], F32R)  # [col-in-chunk, chunk, token]
    q_pool = tc.alloc_tile_pool(name="q", bufs=1)
    q_t = q_pool.tile([P, DC, LQ], F32R)
    normed_pool = tc.alloc_tile_pool(name="normed", bufs=1)
    normed_full = normed_pool.tile([P, DC, L], BF16)

    # =====================================================================
    # Phase A: LN1 over all tokens -> normed_full (feature-major, in place)
    # =====================================================================
    with (
        tc.tile_pool(name="ln1sq", bufs=2) as sq_pool,
        tc.tile_pool(name="ln1coef", bufs=1) as coef_small,
        tc.tile_pool(name="ps_stat", bufs=4, space="PSUM") as ps_stat,
        tc.tile_pool(name="ps_coef", bufs=2, space="PSUM") as ps_coef,
    ):
        for b in range(NBLK):
            blk = normed_full[:, :, b * BLK:(b + 1) * BLK]
            eng = nc.sync if b % 2 == 0 else nc.gpsimd
            eng.dma_start(blk, io["xt_b"][b])
            layer_norm_t(coef_small, ps_stat, ps_coef,
                         [blk[:, c, :] for c in range(DC)], blk, BLK, sq_pool,
                         src3=blk, dst3=blk,
                         add_eng=nc.gpsimd if b % 2 == 0 else nc.vector,
                         ones_lhs=ones_col_bf)

    # =====================================================================
    # Phase B: V (to DRAM scratch) then K^T + cosine-norm, block-pipelined
    # =====================================================================
    with (
        tc.tile_pool(name="wstream", bufs=2) as wstream,
        tc.tile_pool(name="vstage", bufs=3) as vstage,
        tc.tile_pool(name="knorm", bufs=2) as knorm_pool,
        tc.tile_pool(name="ps_mm", bufs=4, space="PSUM") as ps_mm,
        tc.tile_pool(name="ps_nrm", bufs=1, space="PSUM") as ps_nrm,
        tc.tile_pool(name="ps_nbc", bufs=1, space="PSUM") as ps_nbc,
    ):
        # V natural layout, block-major inside each quarter so the first
        # blocks of normed unblock V matmuls early
        for n in range(4):
            wvn = wstream.tile([P, DC, QW], BF16, tag="wv")
            nc.gpsimd.dma_start(wvn, io["wv_t"][n])
            for t in range(KC):
                ps = ps_mm.tile([P, QW], F32, tag="mmv", bufs=2)
                for c in range(DC):
                    _mm(nc, ps, normed_full[:, c, t * P:(t + 1) * P],
                        wvn[:, c, :], c == 0, c == DC - 1)
                stag = vstage.tile([P, 4, DH], BF16, tag="vstage")
                nc.vector.tensor_add(
                    stag, ps.rearrange("p (h d) -> p h d", d=DH),
                    bv_sb[:, n * QW:(n + 1) * QW].rearrange("p (h d) -> p h d",
                                                            d=DH))
                for hp in range(2):
                    nc.gpsimd.dma_start(
                        v_dram[2 * n + hp, :, t, :, :],
                        stag[:, 2 * hp:2 * hp + 2, :])
        # K projection: m-outer so each wk tile is fetched exactly once
        for m in range(DC):
            wkm = wstream.tile([P, DC, P], BF16, tag="wk")
            nc.sync.dma_start(wkm, io["wk_t"][m])
            for b in range(NBLK):
                ps = ps_mm.tile([P, BLK], F32, tag="mm")
                for c in range(DC):
                    _mm(nc, ps, wkm[:, c, :],
                        normed_full[:, c, b * BLK:(b + 1) * BLK], c == 0,
                        c == DC - 1)
                nc.vector.tensor_scalar_add(k_t[:, m, b * BLK:(b + 1) * BLK],
                                            ps, bk_sb[:, m:m + 1])
        # cosine-normalize K blockwise
        for b in range(NBLK):
            nsq = ps_nrm.tile([H, BLK], F32, tag="nsq")
            for m in range(DC):
                ksq = knorm_pool.tile([P, BLK], F32R, tag="ksq")
                nc.scalar.square(ksq, k_t[:, m, b * BLK:(b + 1) * BLK])
                _mm(nc, nsq, selr_sb[:, m, :], ksq, m == 0, m == DC - 1)
            sd = knorm_pool.tile([H, BLK], F32, tag="ksd")
            nc.scalar.activation(sd, nsq,
                                 func=mybir.ActivationFunctionType.Sqrt,
                                 bias=0.0, scale=1.0)
            nc.vector.tensor_scalar_max(sd, sd, NORM_EPS)
            rec = knorm_pool.tile([H, BLK], F32R, tag="krec")
            with nc.allow_low_precision(reason="f32r matmul operand"):
                nc.vector.reciprocal(rec, sd)
            for m in range(DC):
                bc = ps_nbc.tile([P, BLK], F32, tag="nbc")
                _mm(nc, bc, selb_sb[:, m, :], rec, True, True)
                nc.vector.tensor_mul(k_t[:, m, b * BLK:(b + 1) * BLK],
                                     k_t[:, m, b * BLK:(b + 1) * BLK], bc)

    # =====================================================================
    # Phase C: own queries: LN1(own) -> q^T -> cosine-normalize * scaling
    # =====================================================================
    with (
        tc.tile_pool(name="qb", bufs=1) as qb_pool,
        tc.tile_pool(name="qsq", bufs=2) as qsq_pool,
        tc.tile_pool(name="qcoef", bufs=1) as qcoef,
        tc.tile_pool(name="wqstream", bufs=2) as wqstream,
    ):
        normed_own = qb_pool.tile([P, DC, LQ], F32R)
        nc.sync.dma_start(normed_own, xot3)
        with (
            tc.tile_pool(name="ps_stat2", bufs=2, space="PSUM") as ps_stat2,
            tc.tile_pool(name="ps_coef2", bufs=2, space="PSUM") as ps_coef2,
        ):
            layer_norm_t(qcoef, ps_stat2, ps_coef2,
                         [normed_own[:, c, :] for c in range(DC)], normed_own, LQ,
                         qsq_pool)
        with (
            tc.tile_pool(name="ps_mm2", bufs=2, space="PSUM") as ps_mm2,
            tc.tile_pool(name="ps_qn", bufs=2, space="PSUM") as ps_qn,
            tc.tile_pool(name="ps_qbc", bufs=2, space="PSUM") as ps_qbc,
        ):
            for m in range(DC):
                wqm = wqstream.tile([P, DC, P], F32R, tag="wq")
                nc.sync.dma_start(wqm, io["wq_t"][m])
                ps = ps_mm2.tile([P, LQ], F32, tag="mm")
                for c in range(DC):
                    _mm(nc, ps, wqm[:, c, :], normed_own[:, c, :], c == 0,
                        c == DC - 1)
                nc.vector.tensor_scalar_add(q_t[:, m, :], ps, bq_sb[:, m:m + 1])
            # cosine-normalize q (x scaling folded into reciprocal)
            nsq = ps_qn.tile([H, LQ], F32, tag="qnsq")
            for m in range(DC):
                qsq = qsq_pool.tile([P, LQ], F32R, tag="xsq")
                nc.scalar.square(qsq, q_t[:, m, :])
                _mm(nc, nsq, selr_sb[:, m, :], qsq, m == 0, m == DC - 1)
            sd = qcoef.tile([H, LQ], F32, tag="qsd", bufs=2)
            nc.scalar.activation(sd, nsq,
                                 func=mybir.ActivationFunctionType.Sqrt,
                                 bias=0.0, scale=1.0)
            nc.vector.tensor_scalar_max(sd, sd, NORM_EPS)
            rec = qcoef.tile([H, LQ], F32R, tag="qrec", bufs=2)
            with nc.allow_low_precision(reason="f32r matmul operand"):
                nc.vector.reciprocal(rec, sd)
            nc.vector.tensor_scalar_mul(rec, rec, SCALING)
            for m in range(DC):
                bc = ps_qbc.tile([P, LQ], F32, tag="qbc")
                _mm(nc, bc, selb_sb[:, m, :], rec, True, True)
                nc.vector.tensor_mul(q_t[:, m, :], q_t[:, m, :], bc)

    normed_pool.release()

    # =====================================================================
    # Phase D: attention per head-pair, with the out-projection folded in
    # (partial products accumulated into x2acc via DVE)
    # =====================================================================
    with (
        tc.tile_pool(name="exp", bufs=2) as exp_pool,
        tc.tile_pool(name="vsb", bufs=2) as vsb_pool,
        tc.tile_pool(name="rsc", bufs=2) as rsc_pool,
        tc.tile_pool(name="apair", bufs=2) as apair_pool,
        tc.tile_pool(name="wostream", bufs=2) as wostream,
        tc.tile_pool(name="ps_sc", bufs=2, space="PSUM") as ps_sc,
        tc.tile_pool(name="ps_acc", bufs=1, space="PSUM") as ps_acc,
        tc.tile_pool(name="ps_rbc", bufs=1, space="PSUM") as ps_rbc,
        tc.tile_pool(name="ps_op", bufs=2, space="PSUM") as ps_op,
    ):
        for m in range(DC):
            vp = vsb_pool.tile([P, KC, 2, DH + 1], BF16, tag="vp")
            nc.gpsimd.dma_start(vp[:, :, :, 0:DH], v_dram[m])
            nc.vector.tensor_copy(
                vp[:, :, :, DH:DH + 1],
                ones_col_bf.unsqueeze(1).unsqueeze(1)
                .to_broadcast((P, KC, 2, 1)))
            eh = exp_pool.tile([P, KC, 2 * LQ], BF16, tag="exp")
            for kc in range(KC):
                # each head's scores go to a separate PSUM bank: fp32r matmul
                # writes at mid-bank free offsets fault on hardware
                ps = ps_sc.tile([P, 2, 2 * LQ], F32, tag="sc")
                for j in range(2):
                    _mm(nc, ps[:, j, 0:LQ],
                        k_t[j * DH:(j + 1) * DH, m, kc * P:(kc + 1) * P],
                        q_t[j * DH:(j + 1) * DH, m, :], True, True)
                nc.scalar.activation(
                    eh[:, kc, :].rearrange("p (j q) -> p j q", j=2),
                    ps[:, :, 0:LQ],
                    func=mybir.ActivationFunctionType.Exp,
                    bias=lcc_sb[:, kc:kc + 1], scale=1.0)
            attn_pair = apair_pool.tile([P, LQ], BF16, tag="apair")
            for j in range(2):
                acc = ps_acc.tile([DH + 1, LQ], F32, tag="acc")
                for kc in range(KC):
                    _mm(nc, acc, vp[:, kc, j, :],
                        eh[:, kc, j * LQ:(j + 1) * LQ], kc == 0, kc == KC - 1)
                recip = rsc_pool.tile([1, LQ], F32R, tag="recip")
                with nc.allow_low_precision(reason="f32r matmul operand"):
                    nc.vector.reciprocal(recip, acc[DH:DH + 1, :])
                rbc = ps_rbc.tile([DH, LQ], F32, tag="rbc")
                _mm(nc, rbc, ones_1x128[:, 0:DH], recip, True, True)
                rbc_sb = rsc_pool.tile([DH, LQ], F32, tag="rbcsb")
                nc.vector.tensor_copy(rbc_sb, rbc)
                nc.vector.tensor_mul(attn_pair[j * DH:(j + 1) * DH, :],
                                     acc[0:DH, :], rbc_sb)
            # out-projection partial for this pair-chunk of attn
            wom = wostream.tile([P, DC, P], BF16, tag="wo")
            nc.sync.dma_start(wom, io["wo_t"][m])
            for o in range(DC):
                pso = ps_op.tile([P, LQ], F32, tag="op")
                _mm(nc, pso, wom[:, o, :], attn_pair, True, True)
                if m == 0:
                    nc.vector.tensor_copy(x2acc[:, o, :], pso)
                else:
                    nc.vector.tensor_add(x2acc[:, o, :], x2acc[:, o, :], pso)

    q_pool.release()
    kt_pool.release()

    # =====================================================================
    # Phase E: residual -> x2; LN2; FFN (ff2 single-pass, half-packed psum)
    # =====================================================================
    with (
        tc.tile_pool(name="xo2p", bufs=1) as xo2_pool,
        tc.tile_pool(name="ffsq", bufs=2) as ffsq_pool,
        tc.tile_pool(name="ffcoef", bufs=2) as ffcoef,
        tc.tile_pool(name="ht", bufs=1) as ht_pool,
        tc.tile_pool(name="wf1s", bufs=3) as wf1s,
        tc.tile_pool(name="wf2s", bufs=3) as wf2s,
        tc.tile_pool(name="outsb", bufs=2) as outsb_pool,
    ):
        xo2 = xo2_pool.tile([P, DC, LQ], F32R)
        nc.sync.dma_start(xo2, xot3)
        for o in range(DC):
            nc.vector.tensor_scalar_add(x2[:, o, :], x2acc[:, o, :],
                                        bo_sb[:, o:o + 1])
            nc.vector.tensor_add(x2[:, o, :], x2[:, o, :], xo2[:, o, :])
        normed2 = xo2_pool.tile([P, DC, LQ], BF16)
        with (
            tc.tile_pool(name="ps_stat3", bufs=2, space="PSUM") as ps_stat3,
            tc.tile_pool(name="ps_coef3", bufs=2, space="PSUM") as ps_coef3,
        ):
            layer_norm_t(ffcoef, ps_stat3, ps_coef3,
                         [x2[:, c, :] for c in range(DC)], normed2, LQ,
                         ffsq_pool)
        ps_mm3 = tc.alloc_tile_pool(name="ps_mm3", bufs=3, space="PSUM")
        ps_ff2 = tc.alloc_tile_pool(name="ps_ff2", bufs=4, space="PSUM")
        h_t = ht_pool.tile([P, FC, LQ], BF16)
        for f in range(FC):
            wf1m = wf1s.tile([P, DC, P], BF16, tag="wf1")
            weng = nc.sync if f % 2 == 0 else nc.gpsimd
            weng.dma_start(wf1m, io["wf1_t"][f])
            ps = ps_mm3.tile([P, LQ], F32, tag="mm")
            for c in range(DC):
                _mm(nc, ps, wf1m[:, c, :], normed2[:, c, :], c == 0, c == DC - 1)
            nc.scalar.activation(h_t[:, f, :], ps, func=GELU_FUNC,
                                 bias=bf1_sb[:, f:f + 1], scale=1.0)
        # ff2: f-outer accumulation in two 4-output passes; pass 1 pipelines
        # with ff1 chunk by chunk
        for g in range(2):
            accs = [ps_ff2.tile([P, LQ], F32, tag="ff2acc",
                                name=f"ff2acc_{g}_{i}") for i in range(4)]
            for f in range(FC):
                wf2m = wf2s.tile([P, 4, P], BF16, tag="wf2")
                weng2 = nc.gpsimd if f % 2 == 0 else nc.sync
                weng2.dma_start(wf2m, io["wf2_t"][g][f])
                for i in range(4):
                    _mm(nc, accs[i], wf2m[:, i, :], h_t[:, f, :],
                        f == 0, f == FC - 1)
            for i in range(4):
                mcol = g * 4 + i
                osb = outsb_pool.tile([P, LQ], F32, tag="osb")
                nc.vector.tensor_scalar_add(osb, accs[i], bf2_sb[:, mcol:mcol + 1])
                nc.vector.tensor_add(osb, osb, x2[:, mcol, :])
                nc.sync.dma_start(out3[:, mcol, :], osb)
        ps_ff2.release()
        ps_mm3.release()

    x2_pool.release()
    vdram_pool.release()
    singles.release()


_CACHED = {}


def build_k(k=1):
    """Build a NEFF with the kernel body emitted k times back-to-back.

    k=1 is the production kernel; k>1 exists so a timing harness can
    measure steady-state per-execution time on-device (the repetitions
    are data-independent re-runs writing the same output)."""
    if k not in _CACHED:
        nc = bacc.Bacc("TRN2", target_bir_lowering=False, debug=False)
        with tile.TileContext(nc) as tc:
            io = declare_io(nc)
            for r in range(k):
                body(tc, io, sfx=f"_{r}" if k > 1 else "")
        nc.compile()
        _CACHED[k] = nc
    return _CACHED[k]


def build():
    return build_k(1)


def _onesc_matrix():
    o = np.zeros((P, 3), np.float32)
    o[:, 0] = 1.0
    o[0:DH, 1] = 1.0
    o[DH:P, 2] = 1.0
    return o


def _selr_matrix():
    # [P, DC*H]: selr[p, m*16+h] = 1 iff h == 2m + (p >= 64)
    s = np.zeros((P, DC, H), np.float32)
    for m in range(DC):
        s[0:DH, m, 2 * m] = 1.0
        s[DH:P, m, 2 * m + 1] = 1.0
    return np.ascontiguousarray(s.reshape(P, P))


def _selb_matrix():
    # [H, DC*P]: selb[h, m*128+p] = 1 iff h == 2m + (p >= 64)
    s = np.zeros((H, DC, P), np.float32)
    for m in range(DC):
        s[2 * m, m, 0:DH] = 1.0
        s[2 * m + 1, m, DH:P] = 1.0
    return np.ascontiguousarray(s.reshape(H, DC * P))


def _pcn(w):
    """[C*P, n] -> contiguous [P, C, n] with row c*P+p landing at [p, c]."""
    n = w.shape[1]
    return np.ascontiguousarray(w.reshape(-1, P, n).transpose(1, 0, 2))


def prep_inputs(inputs):
    """Host-side preprocessing: transpose x, split/fold weights, pack all
    inputs into two flat tensors (see _R_SEGS/_F_SEGS)."""
    f = np.float32
    x = np.asarray(inputs["x"], f)
    lcc = np.asarray(inputs["lcc_values"], f)
    w_qkv = np.asarray(inputs["w_qkv"], f)
    b_qkv = np.asarray(inputs["b_qkv"], f)
    ln1_g = np.asarray(inputs["ln1_g"], f)
    ln1_b = np.asarray(inputs["ln1_b"], f)
    ln2_g = np.asarray(inputs["ln2_g"], f)
    ln2_b = np.asarray(inputs["ln2_b"], f)
    w_ff1 = np.asarray(inputs["w_ff1"], f)
    b_ff1 = np.asarray(inputs["b_ff1"], f)

    def chunked(b):  # [D] -> [128, DC] with chunk c in column c
        return np.ascontiguousarray(b.reshape(-1, P).T)

    def tiled(w, ncols):  # [D, n] -> [n/ncols, P, DC, ncols] tile-major
        return np.ascontiguousarray(
            w.reshape(DC, P, w.shape[1] // ncols, ncols).transpose(2, 1, 0, 3))

    xt = np.ascontiguousarray(x.T)
    xt3 = _pcn(xt)
    wf2 = np.asarray(inputs["w_ff2"], f)
    segs_r = {
        "xot3": None,  # per-core
        "wq_t": tiled(ln1_g[:, None] * w_qkv[:, 0:D], P),
        "selr3": _selr_matrix().reshape(P, DC, H),
        "selb3": _selb_matrix().reshape(H, DC, P),
        "onesc": _onesc_matrix(),
        "ones1r": np.ones((1, P), f),
    }
    segs_h = {
        "xt_b": np.ascontiguousarray(
            xt3.reshape(P, DC, NBLK, BLK).transpose(2, 0, 1, 3)),
        "wk_t": tiled(ln1_g[:, None] * w_qkv[:, D:2 * D], P),
        "wv_t": tiled(ln1_g[:, None] * w_qkv[:, 2 * D:3 * D], QW),
        "wo_t": np.asarray(inputs["w_out"], f).reshape(DC, P, DC, P),
        "wf1_t": tiled(ln2_g[:, None] * w_ff1, P),
        "wf2_t": np.ascontiguousarray(
            wf2.reshape(FC, P, 2, 4, P).transpose(2, 0, 1, 3, 4)),
        "onesc_h": _onesc_matrix(),
    }
    segs_f = {
        "bq": chunked(b_qkv[0:D] + ln1_b @ w_qkv[:, 0:D]),
        "bk": chunked(b_qkv[D:2 * D] + ln1_b @ w_qkv[:, D:2 * D]),
        "bv": np.ascontiguousarray(b_qkv[2 * D:3 * D] + ln1_b @ w_qkv[:, 2 * D:3 * D]),
        "bo": chunked(np.asarray(inputs["b_out"], f)),
        "bf1": chunked(b_ff1 + ln2_b @ w_ff1),
        "bf2": chunked(np.asarray(inputs["b_ff2"], f)),
        "lcck": np.ascontiguousarray((lcc * (0.5 * LCC)).reshape(KC, P).T),
    }

    pack_r = np.empty((_R_TOT,), f)
    for name, shape in _R_SEGS:
        if name == "xot3":
            continue
        seg = segs_r[name]
        assert seg.shape == shape, (name, seg.shape, shape)
        pack_r[_R_OFFS[name]:_R_OFFS[name] + seg.size] = seg.ravel()
    bf = mybir.dt.np(BF16)
    pack_h = np.empty((_H_TOT,), bf)
    for name, shape in _H_SEGS:
        seg = segs_h[name]
        assert seg.shape == shape, (name, seg.shape, shape)
        pack_h[_H_OFFS[name]:_H_OFFS[name] + seg.size] = \
            seg.ravel().astype(bf)
    pack_f = np.empty((_F_TOT,), f)
    for name, shape in _F_SEGS:
        seg = segs_f[name]
        assert seg.shape == shape, (name, seg.shape, shape)
        pack_f[_F_OFFS[name]:_F_OFFS[name] + seg.size] = seg.ravel()

    o, sz = _R_OFFS["xot3"], P * DC * LQ
    in_maps = []
    for c in range(NCORES):
        pr = pack_r.copy()
        pr[o:o + sz] = np.ascontiguousarray(
            xt3[:, :, c * LQ:(c + 1) * LQ]).ravel()
        in_maps.append({"pack_r": pr, "pack_h": pack_h, "pack_f": pack_f})
    return in_maps


def kernel(**inputs):
    nc = build()
    in_maps = prep_inputs(inputs)
    res = run_bass_kernel_spmd(nc, in_maps, core_ids=list(range(NCORES)))
    out = np.concatenate([res.results[c]["out_t"] for c in range(NCORES)], axis=1)
    return np.ascontiguousarray(out.T).astype(np.float32)

